# revision 1
# baseline (speedup 1.0000x reference)
"""CSAEncoder Trainium2 kernel: 3-branch cross-attention + concat DoubleConv.

Sharding (8 cores): 2 batch groups x 4 tensor ranks.
Core c: batch b = c // 4, rank g = c % 4.
  - Attention: core computes heads [4g, 4g+4) of all 3 branches for batch b
    (a contiguous 128-channel slab of each branch's output).
  - conv1 computed as partial sums over the core's local 384 input channels
    for ALL 512 output channels; one chunked AllReduce(add) within the 4-core
    batch group gives every rank the full conv1 pre-BN output.
  - conv2 computed locally: full 512-channel contraction, only the core's own
    128 output channels. No second collective.
Host assembles the full (2, 512, 32, 32) output from the 8 per-core slabs.
"""

import os
import sys

import ml_dtypes
import numpy as np

for _p in ("/opt/trn_rl_repo",):
    if _p not in sys.path and os.path.isdir(_p):
        sys.path.insert(0, _p)

import concourse.bass as bass
import concourse.mybir as mybir
import concourse.tile as tile
from concourse import bacc
from concourse.bass_utils import run_bass_kernel_spmd

F32 = mybir.dt.float32
BF16 = mybir.dt.bfloat16
AF = mybir.ActivationFunctionType

B, C, H, W, HEADS = 2, 512, 32, 32, 16
D = C // HEADS            # 32
S = H * W                 # 1024
EPS = 1e-5
ISQD = 1.0 / np.sqrt(D)   # folded into the exp activation
NCORES = 8
GROUPS = [[0, 1, 2, 3], [4, 5, 6, 7]]
HP = W + 2                # padded row stride (34)


def build_nc():
    nc = bacc.Bacc(None, target_bir_lowering=False)

    # ---- per-core external inputs -------------------------------------
    x4_d = nc.declare_dram_parameter("x4", [4, 128, S], BF16, isOutput=False)
    oth_d = nc.declare_dram_parameter("oth", [2, 4, 128, S], BF16, isOutput=False)
    wqT_d = nc.declare_dram_parameter("wqT", [3, 4, 128, 128], BF16, isOutput=False)
    wkT_d = nc.declare_dram_parameter("wkT", [3, 4, 128, 128], BF16, isOutput=False)
    wvoT_d = nc.declare_dram_parameter("wvoT", [4, 128, 384], BF16, isOutput=False)
    dvec_d = nc.declare_dram_parameter("dvec", [128, 9], F32, isOutput=False)
    wobv_d = nc.declare_dram_parameter("wobv", [1, 384], F32, isOutput=False)
    c1wT_d = nc.declare_dram_parameter("c1wT", [3, 4, 128, 9, 128], BF16, isOutput=False)
    c2wT_d = nc.declare_dram_parameter("c2wT", [4, 128, 9, 128], BF16, isOutput=False)
    avec_d = nc.declare_dram_parameter("avec", [128, 10], F32, isOutput=False)
    out_d = nc.declare_dram_parameter("out", [128, S], F32, isOutput=True)

    with tile.TileContext(nc) as tc:
        import contextlib

        ctx = contextlib.ExitStack()
        with ctx:
            const = ctx.enter_context(tc.tile_pool(name="const", bufs=1))
            kq = ctx.enter_context(tc.tile_pool(name="kq", bufs=1))
            xtp = ctx.enter_context(tc.tile_pool(name="xtp", bufs=1))
            stg = ctx.enter_context(tc.tile_pool(name="stg", bufs=4))
            scps = ctx.enter_context(tc.tile_pool(name="scps", bufs=2, space="PSUM"))
            smps = ctx.enter_context(tc.tile_pool(name="smps", bufs=4, space="PSUM"))
            dram = ctx.enter_context(tc.tile_pool(name="dram", bufs=1, space="DRAM"))
            dramw = ctx.enter_context(tc.tile_pool(name="dramw", bufs=4, space="DRAM"))

            # ---- activations first (highest DMA priority) ----------------
            x_sb = const.tile([128, 4, S], BF16)
            oth_sb = const.tile([128, 2, 4, S], BF16)
            for ks in range(4):
                nc.sync.dma_start(out=x_sb[:, ks, :], in_=x4_d[ks])
            for o in range(2):
                for ks in range(4):
                    nc.sync.dma_start(out=oth_sb[:, o, ks, :], in_=oth_d[o, ks])

            # ---- constants to SBUF ---------------------------------------
            wq_sb = const.tile([128, 3, 4, 128], BF16)
            wk_sb = const.tile([128, 3, 4, 128], BF16)
            wvo_sb = const.tile([128, 4, 384], BF16)
            for i in range(3):
                for ks in range(4):
                    nc.sync.dma_start(out=wq_sb[:, i, ks, :], in_=wqT_d[i, ks])
                    nc.sync.dma_start(out=wk_sb[:, i, ks, :], in_=wkT_d[i, ks])
            for ks in range(4):
                nc.sync.dma_start(out=wvo_sb[:, ks, :], in_=wvoT_d[ks])

            # Small consts: DMA to staging, then re-own on the consuming
            # engine (DVE / ACT) so consumers need no cross-engine const wait.
            dvec_st = const.tile([128, 9], F32)
            nc.gpsimd.dma_start(out=dvec_st, in_=dvec_d[:])
            wobv_st = const.tile([128, 384], F32)
            nc.gpsimd.dma_start(out=wobv_st, in_=wobv_d[:].partition_broadcast(128))
            avec_st = const.tile([128, 10], F32)
            nc.gpsimd.dma_start(out=avec_st, in_=avec_d[:])
            dvec = const.tile([128, 9], F32)
            nc.vector.tensor_copy(dvec, dvec_st)
            wobv_sb = const.tile([128, 384], F32)
            nc.vector.tensor_copy(wobv_sb, wobv_st)
            avec = const.tile([128, 10], F32)
            nc.scalar.activation(out=avec, in_=avec_st, func=AF.Copy)
            bqv_sb = dvec[:, 0:3]
            bkv_sb = dvec[:, 3:6]
            xtb_sb = dvec[:, 6:9]

            # xt (attention output) slabs + h1 slabs, zero-padded 34x34
            xt_sl = []
            for i in range(3):
                t = xtp.tile([128, HP, HP], BF16, name=f"xt{i}")
                nc.vector.memset(t, 0.0)
                xt_sl.append(t)
            h1_sl = []
            for k in range(4):
                t = xtp.tile([128, HP, HP], BF16, name=f"h1{k}")
                nc.vector.memset(t, 0.0)
                h1_sl.append(t)
            # conv1 partial-sum accumulators (512 out ch as 4 m-tiles)
            acc1 = [xtp.tile([128, S], F32, name=f"acc1{m}") for m in range(4)]

            # Semaphore warmers: absorb const-DMA + memset waits into each
            # engine's observed clock so later compute ops need <=1 wait.
            warm = const.tile([128, 1], F32)
            nc.vector.tensor_copy(warm, dvec[:, 0:1])
            warm2 = const.tile([128, 1], F32)
            nc.scalar.activation(out=warm2, in_=warm, func=AF.Copy)

            # k/q per branch (with biases added), uT tiles
            k_sb = kq.tile([128, 3, S], BF16)
            q_sb = kq.tile([128, 3, S], BF16)
            uT = [kq.tile([128, 3, 4, 33], BF16, name=f"uT{t}") for t in range(8)]

            # ---- phase P: projections ------------------------------------
            if True:
                qsrc = [oth_sb[:, 0], x_sb, oth_sb[:, 1]]
                for i in range(3):
                    k_ps = scps.tile([128, S], F32, name="kq_ps", tag="sc")
                    for s in range(2):
                        for ks in range(4):
                            nc.tensor.matmul(
                                k_ps[:, 512 * s : 512 * (s + 1)],
                                lhsT=wk_sb[:, i, ks, :],
                                rhs=x_sb[:, ks, 512 * s : 512 * (s + 1)],
                                start=(ks == 0),
                                stop=(ks == 3),
                            )
                    nc.vector.tensor_scalar_add(k_sb[:, i, :], k_ps, bkv_sb[:, i : i + 1])

                    q_ps = scps.tile([128, S], F32, name="kq_ps2", tag="sc")
                    for s in range(2):
                        for ks in range(4):
                            nc.tensor.matmul(
                                q_ps[:, 512 * s : 512 * (s + 1)],
                                lhsT=wq_sb[:, i, ks, :],
                                rhs=qsrc[i][:, ks, 512 * s : 512 * (s + 1)],
                                start=(ks == 0),
                                stop=(ks == 3),
                            )
                    nc.vector.tensor_scalar_add(q_sb[:, i, :], q_ps, bqv_sb[:, i : i + 1])

                for t in range(8):
                    u_ps = smps.tile([128, 384], F32, name="u_ps", tag="sm")
                    for ks in range(4):
                        nc.tensor.matmul(
                            u_ps,
                            lhsT=x_sb[:, ks, 128 * t : 128 * (t + 1)],
                            rhs=wvo_sb[:, ks, :],
                            start=(ks == 0),
                            stop=(ks == 3),
                        )
                    nc.vector.memset(uT[t][:, :, :, 32:33], 1.0)
                    nc.vector.tensor_add(
                        uT[t][:, :, :, 0:32],
                        u_ps.rearrange("p (i h d) -> p i h d", i=3, h=4),
                        wobv_sb.rearrange("p (i h d) -> p i h d", i=3, h=4),
                    )

            # ---- conv weights (emitted after projp freed) ----------------
            convw = ctx.enter_context(tc.tile_pool(name="convw", bufs=1))
            pt = ctx.enter_context(tc.tile_pool(name="pt", bufs=16))
            c1w_sb = [
                [convw.tile([128, 9, 128], BF16, name=f"c1w{i}_{m}") for m in range(4)]
                for i in range(3)
            ]
            for i in range(3):
                for m in range(4):
                    nc.sync.dma_start(out=c1w_sb[i][m], in_=c1wT_d[i, m])
            c2w_sb = [convw.tile([128, 9, 128], BF16, name=f"c2w{k}") for k in range(4)]
            for k in range(4):
                nc.sync.dma_start(out=c2w_sb[k], in_=c2wT_d[k])

            def conv1_block(i, m, n):
                """Partial conv1 for xt slab i, out m-tile, spatial half n,
                accumulated into acc1[m]."""
                ps = smps.tile([128, 512], F32, name="cv", tag="sm")
                for dy in range(3):
                    for dx in range(3):
                        nc.tensor.matmul(
                            ps,
                            lhsT=c1w_sb[i][m][:, dy * 3 + dx, :],
                            rhs=xt_sl[i][:, 16 * n + dy : 16 * n + dy + 16, dx : dx + 32],
                            start=(dy == 0 and dx == 0),
                            stop=(dy == 2 and dx == 2),
                        )
                dst = acc1[m][:, 512 * n : 512 * (n + 1)]
                if i == 0:
                    nc.vector.tensor_copy(dst, ps)
                else:
                    nc.vector.tensor_add(dst, ps, dst)

            def attention(i, pr):
                heads = (2 * pr, 2 * pr + 1)
                pts = {}
                for t in range(8):
                    scs = {}
                    for h in heads:
                        sc = scps.tile([128, S], F32, name="sc", tag="sc")
                        p0 = 32 * h
                        for s in range(2):
                            nc.tensor.matmul(
                                sc[:, 512 * s : 512 * (s + 1)],
                                lhsT=k_sb[p0 : p0 + 32, i, 128 * t : 128 * (t + 1)],
                                rhs=q_sb[p0 : p0 + 32, i, 512 * s : 512 * (s + 1)],
                                start=True,
                                stop=True,
                                tile_position=(p0, 0),
                            )
                        scs[h] = sc
                    for h in heads:
                        ptt = pt.tile([128, S], BF16, name="ptt")
                        nc.scalar.activation(
                            out=ptt, in_=scs[h], func=AF.Exp, scale=float(ISQD)
                        )
                        pts[(h, t)] = ptt
                for h in heads:
                    for s in range(2):
                        y = smps.tile([33, 512], F32, name="y", tag="sm")
                        for t in range(8):
                            nc.tensor.matmul(
                                y,
                                lhsT=uT[t][:, i, h, :],
                                rhs=pts[(h, t)][:, 512 * s : 512 * (s + 1)],
                                start=(t == 0),
                                stop=(t == 7),
                            )
                        rc = stg.tile([1, 512], F32, name="rc")
                        nc.vector.reciprocal(rc, y[32:33, :])
                        rcd = dramw.tile([1, 512], F32, name="rcd", tag="rcd")
                        nc.gpsimd.dma_start(out=rcd, in_=rc)
                        rcb = stg.tile([32, 512], F32, name="rcb")
                        nc.gpsimd.dma_start(out=rcb, in_=rcd[:].partition_broadcast(32))
                        tmp = stg.tile([32, 512], F32, name="tmp")
                        nc.vector.tensor_mul(tmp, y[0:32, :], rcb)
                        p0 = 32 * h
                        nc.vector.tensor_scalar_add(
                            xt_sl[i][p0 : p0 + 32, 1 + 16 * s : 17 + 16 * s, 1:33],
                            tmp.rearrange("p (a b) -> p a b", b=32),
                            xtb_sb[p0 : p0 + 32, i : i + 1],
                        )

            # ---- phase A: attention with conv1 interleaved ---------------
            # conv1 for xt slab i-1 is emitted between the pairs of branch i
            # so PE fills the gaps while ACT grinds through the exps.
            for i in range(3):
                attention(i, 0)
                if i > 0:
                    for m in range(4):
                        conv1_block(i - 1, m, 0)
                attention(i, 1)
                if i > 0:
                    for m in range(4):
                        conv1_block(i - 1, m, 1)
            for m in range(4):
                for n in range(2):
                    conv1_block(2, m, n)

            # ---- phase C: AllReduce conv1, BN1, conv2, BN2, out ----------
            partial1 = [dram.tile([256, S], F32, name=f"partial1{a}") for a in range(2)]
            art = [dram.tile([256, S], F32, name=f"art{a}") for a in range(2)]

            def ar_chunk(a):
                nc.gpsimd.collective_compute(
                    "AllReduce",
                    mybir.AluOpType.add,
                    replica_groups=GROUPS,
                    ins=[partial1[a][:]],
                    outs=[art[a][:]],
                )

            for m in range(4):
                nc.sync.dma_start(
                    out=partial1[m // 2][(m % 2) * 128 : (m % 2) * 128 + 128, :],
                    in_=acc1[m],
                )
                if m == 1:
                    ar_chunk(0)
            ar_chunk(1)

            arraw = stg.tile([128, 4, S], F32, name="arraw", bufs=1)
            oout = stg.tile([128, S], F32, name="oout", bufs=1)
            ps2 = [smps.tile([128, 512], F32, name=f"cv2_{n}", tag="sm") for n in range(2)]

            def h1_chunk(a):
                nc.gpsimd.dma_start(
                    out=arraw[:, 2 * a : 2 * a + 2, :],
                    in_=art[a][:].rearrange("(k p) s -> p k s", p=128),
                )
                for k in (2 * a, 2 * a + 1):
                    nc.scalar.activation(
                        out=h1_sl[k][:, 1:33, 1:33],
                        in_=arraw[:, k, :].rearrange("p (a b) -> p a b", b=32),
                        func=AF.Relu,
                        bias=avec[:, 4 + k : 5 + k],
                        scale=avec[:, k : k + 1],
                    )

            def conv2_half(a):
                # accumulate k-slabs 2a, 2a+1 into both spatial halves
                for n in range(2):
                    for k in (2 * a, 2 * a + 1):
                        for dy in range(3):
                            for dx in range(3):
                                nc.tensor.matmul(
                                    ps2[n],
                                    lhsT=c2w_sb[k][:, dy * 3 + dx, :],
                                    rhs=h1_sl[k][
                                        :, 16 * n + dy : 16 * n + dy + 16, dx : dx + 32
                                    ],
                                    start=(k == 0 and dy == 0 and dx == 0),
                                    stop=(k == 3 and dy == 2 and dx == 2),
                                )

            h1_chunk(0)
            conv2_half(0)   # overlaps AR chunk 1
            h1_chunk(1)
            conv2_half(1)
            for n in range(2):
                nc.scalar.activation(
                    out=oout[:, 512 * n : 512 * (n + 1)],
                    in_=ps2[n],
                    func=AF.Relu,
                    bias=avec[:, 9:10],
                    scale=avec[:, 8:9],
                )
            nc.sync.dma_start(out=out_d[:], in_=oout)

    nc.finalize()
    return nc


def _f(x):
    return np.ascontiguousarray(x, dtype=np.float32)


def _bf(x):
    return np.ascontiguousarray(np.asarray(x, dtype=np.float32).astype(ml_dtypes.bfloat16))


def prepare_core_inputs(inp):
    """Build the 8 per-core input dicts from the full-problem inputs."""
    inp = {k: np.asarray(v, dtype=np.float64) for k, v in inp.items()}
    x = inp["x"].reshape(B, C, S)
    xp = inp["x_prev"].reshape(B, C, S)
    xn = inp["x_next"].reshape(B, C, S)

    bn1s_full = inp["bn1g"] / np.sqrt(inp["bn1v"] + EPS)
    bn1b_full = inp["bn1b"] - inp["bn1m"] * bn1s_full
    bn2s_full = inp["bn2g"] / np.sqrt(inp["bn2v"] + EPS)
    bn2b_full = inp["bn2b"] - inp["bn2m"] * bn2s_full

    per_g = []
    for g in range(4):
        sl = slice(128 * g, 128 * (g + 1))
        wqT = np.stack(
            [
                np.stack([inp["Wq"][i][sl, 128 * k : 128 * (k + 1)].T for k in range(4)])
                for i in range(3)
            ]
        )
        wkT = np.stack(
            [
                np.stack([inp["Wk"][i][sl, 128 * k : 128 * (k + 1)].T for k in range(4)])
                for i in range(3)
            ]
        )
        bqv = np.stack([inp["bq"][i][sl] for i in range(3)], axis=1)
        bkv = np.stack([inp["bk"][i][sl] for i in range(3)], axis=1)

        att_s = np.stack(
            [inp["bng"][i][sl] / np.sqrt(inp["bnv"][i][sl] + EPS) for i in range(3)]
        )  # (3,128)
        xtb = np.stack(
            [
                inp["bnb"][i][sl] + (inp["bo"][i][sl] - inp["bnm"][i][sl]) * att_s[i]
                for i in range(3)
            ],
            axis=1,
        )  # (128,3)

        wvo_rows = []
        wobv_row = []
        for i in range(3):
            for hl in range(4):
                hg = 4 * g + hl
                wv_h = inp["Wv"][i][32 * hg : 32 * (hg + 1), :]  # (32, 512)
                bv_h = inp["bv"][i][32 * hg : 32 * (hg + 1)]
                wo_h = inp["Wo"][i, hg]  # (32, 32)
                sc = att_s[i][32 * hl : 32 * (hl + 1)]  # (32,)
                wvo_rows.append(sc[:, None] * (wo_h @ wv_h))
                wobv_row.append(sc * (wo_h @ bv_h))
        wvo_all = np.concatenate(wvo_rows, axis=0)  # (384, 512)
        wobv = np.concatenate(wobv_row)[None, :]  # (1, 384)
        wvoT = np.stack([wvo_all[:, 128 * k : 128 * (k + 1)].T for k in range(4)])

        c1wT = np.stack(
            [
                np.stack(
                    [
                        inp["c1w"][
                            128 * m : 128 * (m + 1),
                            512 * i + 128 * g : 512 * i + 128 * (g + 1),
                        ]
                        .transpose(1, 2, 3, 0)
                        .reshape(128, 9, 128)
                        for m in range(4)
                    ]
                )
                for i in range(3)
            ]
        )
        c2wT = np.stack(
            [
                inp["c2w"][sl, 128 * k : 128 * (k + 1)]
                .transpose(1, 2, 3, 0)
                .reshape(128, 9, 128)
                for k in range(4)
            ]
        )
        avec = np.concatenate(
            [
                bn1s_full.reshape(4, 128).T,
                bn1b_full.reshape(4, 128).T,
                bn2s_full[sl][:, None],
                bn2b_full[sl][:, None],
            ],
            axis=1,
        )  # (128, 10)

        per_g.append(
            dict(
                wqT=_bf(wqT), wkT=_bf(wkT), wvoT=_bf(wvoT),
                wobv=_f(wobv), c1wT=_bf(c1wT), c2wT=_bf(c2wT),
                dvec=_f(np.concatenate([bqv, bkv, xtb], axis=1)),
                avec=_f(avec),
            )
        )

    in_maps = []
    for c in range(NCORES):
        b, g = c // 4, c % 4
        d = dict(per_g[g])
        d["x4"] = _bf(x[b].reshape(4, 128, S))
        d["oth"] = _bf(np.stack([xn[b].reshape(4, 128, S), xp[b].reshape(4, 128, S)]))
        in_maps.append(d)
    return in_maps


_NC_CACHE = {}


def get_nc():
    if "nc" not in _NC_CACHE:
        _NC_CACHE["nc"] = build_nc()
    return _NC_CACHE["nc"]


def assemble(results):
    out = np.zeros((B, C, H, W), dtype=np.float32)
    for c in range(NCORES):
        b, g = c // 4, c % 4
        out[b, 128 * g : 128 * (g + 1)] = results[c]["out"].reshape(128, H, W)
    return out


def kernel(**inputs):
    nc = get_nc()
    in_maps = prepare_core_inputs(inputs)
    res = run_bass_kernel_spmd(nc, in_maps, list(range(NCORES)))
    return assemble(res.results)



# revision 22
# speedup vs baseline: 1.1129x; 1.1129x over previous
"""CSAEncoder Trainium2 kernel: 3-branch cross-attention + concat DoubleConv.

Sharding (8 cores): 2 batch groups x 4 tensor ranks.
Core c: batch b = c // 4, rank g = c % 4.
  - Attention: core computes heads [4g, 4g+4) of all 3 branches for batch b
    (a contiguous 128-channel slab of each branch's output).
  - After branch i's attention output (xt slab, bf16, zero-padded 34x34) is
    done, it is AllGather'd within the 4-core batch group; conv1 then runs
    fully local: own 128 output channels contracted over all 1536 gathered
    input channels.  Branch-i conv1 work is interleaved into branch-(i+1)
    attention as PE filler, so only branch 2's chunk is in the tail.
  - h1 = relu(bn1(conv1)) for the own 128 channels, AllGather'd in 2
    spatial chunks overlapping conv2; conv2 = full 512-channel contraction,
    own 128 output channels.
Host assembles the full (2, 512, 32, 32) output from the 8 per-core slabs.

Softmax normalization: the y matmul packs two heads into one [66,512] PSUM
tile with an appended ones/bias column per head arranged so both softmax
denominators land on adjacent rows (32, 33); reciprocal_approx_fast on DVE,
partition_broadcast on GpSimd, one DVE multiply writes the BN'd output (the
BN bias is folded into the v/o projection bias on the host).
"""

import os
import sys

import ml_dtypes
import numpy as np

for _p in ("/opt/trn_rl_repo",):
    if _p not in sys.path and os.path.isdir(_p):
        sys.path.insert(0, _p)

import concourse.bass as bass
import concourse.mybir as mybir
import concourse.tile as tile
from concourse import bacc
from concourse.bass_utils import run_bass_kernel_spmd

F32 = mybir.dt.float32
BF16 = mybir.dt.bfloat16
AF = mybir.ActivationFunctionType

B, C, H, W, HEADS = 2, 512, 32, 32, 16
D = C // HEADS            # 32
S = H * W                 # 1024
EPS = 1e-5
ISQD = 1.0 / np.sqrt(D)   # folded into the exp activation
NCORES = 8
GROUPS = [[0, 1, 2, 3], [4, 5, 6, 7]]
HP = W + 2                # padded row stride (34)
# conv1 output-row blocks (free dim <= 512 fp32 PSUM): 11/11/10 rows
C1ROWS = [(0, 11), (11, 11), (22, 10)]
# h1 allgather chunks in padded rows: [0,18) and [18,34)
H1CH = [(0, 18), (18, 16)]
DEBUG = int(os.environ.get("KDEBUG", "0"))


def build_nc():
    nc = bacc.Bacc(None, target_bir_lowering=False)

    # ---- per-core external inputs -------------------------------------
    x4_d = nc.declare_dram_parameter("x4", [4, 128, S], BF16, isOutput=False)
    oth_d = nc.declare_dram_parameter("oth", [2, 4, 128, S], BF16, isOutput=False)
    wqT_d = nc.declare_dram_parameter("wqT", [3, 4, 128, 128], BF16, isOutput=False)
    wkT_d = nc.declare_dram_parameter("wkT", [3, 4, 128, 128], BF16, isOutput=False)
    wvoT_d = nc.declare_dram_parameter("wvoT", [4, 128, 384], BF16, isOutput=False)
    dvec_d = nc.declare_dram_parameter("dvec", [128, 6], F32, isOutput=False)
    wobv_d = nc.declare_dram_parameter("wobv", [1, 384], F32, isOutput=False)
    c1wT_d = nc.declare_dram_parameter("c1wT", [3, 4, 128, 9, 128], BF16, isOutput=False)
    c2wT_d = nc.declare_dram_parameter("c2wT", [4, 128, 9, 128], BF16, isOutput=False)
    avec_d = nc.declare_dram_parameter("avec", [128, 4], F32, isOutput=False)
    out_d = nc.declare_dram_parameter("out", [128, S], F32, isOutput=True)

    with tile.TileContext(nc) as tc:
        import contextlib

        ctx = contextlib.ExitStack()
        with ctx:
            const = ctx.enter_context(tc.tile_pool(name="const", bufs=1))
            kq = ctx.enter_context(tc.tile_pool(name="kq", bufs=1))
            xtp = ctx.enter_context(tc.tile_pool(name="xtp", bufs=1))
            stg = ctx.enter_context(tc.tile_pool(name="stg", bufs=4))
            scps = ctx.enter_context(tc.tile_pool(name="scps", bufs=2, space="PSUM"))
            yps = ctx.enter_context(tc.tile_pool(name="yps", bufs=3, space="PSUM"))
            cvps = ctx.enter_context(tc.tile_pool(name="cvps", bufs=1, space="PSUM"))
            dram = ctx.enter_context(tc.tile_pool(name="dram", bufs=1, space="DRAM"))
            dramw = ctx.enter_context(tc.tile_pool(name="dramw", bufs=4, space="DRAM"))
            pt = ctx.enter_context(tc.tile_pool(name="pt", bufs=20))

            # ---- first-needed DMAs (weights for proj 0, x, q-source) -----
            wq_sb = const.tile([128, 3, 4, 128], BF16)
            wk_sb = const.tile([128, 3, 4, 128], BF16)
            x_sb = const.tile([128, 4, S], BF16)
            oth_sb = const.tile([128, 2, 4, S], BF16)
            wvo_sb = const.tile([128, 4, 384], BF16)
            for ks in range(4):
                nc.sync.dma_start(out=wk_sb[:, 0, ks, :], in_=wkT_d[0, ks])
                nc.sync.dma_start(out=wq_sb[:, 0, ks, :], in_=wqT_d[0, ks])
            for ks in range(4):
                nc.sync.dma_start(out=x_sb[:, ks, :], in_=x4_d[ks])
            for ks in range(4):
                nc.sync.dma_start(out=oth_sb[:, 0, ks, :], in_=oth_d[0, ks])
            for ks in range(4):
                nc.sync.dma_start(out=wvo_sb[:, ks, :], in_=wvoT_d[ks])
            for i in range(1, 3):
                for ks in range(4):
                    nc.sync.dma_start(out=wk_sb[:, i, ks, :], in_=wkT_d[i, ks])
                    nc.sync.dma_start(out=wq_sb[:, i, ks, :], in_=wqT_d[i, ks])
            for ks in range(4):
                nc.sync.dma_start(out=oth_sb[:, 1, ks, :], in_=oth_d[1, ks])

            # Small consts: DMA to staging, then re-own on the consuming
            # engine (DVE / ACT) so consumers need no cross-engine const wait.
            dvec_st = const.tile([128, 6], F32)
            nc.gpsimd.dma_start(out=dvec_st, in_=dvec_d[:])
            wobv_st = const.tile([128, 384], F32)
            nc.gpsimd.dma_start(out=wobv_st, in_=wobv_d[:].partition_broadcast(128))
            avec_st = const.tile([128, 4], F32)
            nc.gpsimd.dma_start(out=avec_st, in_=avec_d[:])
            dvec = const.tile([128, 6], F32)
            nc.vector.tensor_copy(dvec, dvec_st)
            wobv_sb = const.tile([128, 384], F32)
            nc.vector.tensor_copy(wobv_sb, wobv_st)
            avec = const.tile([128, 4], F32)
            nc.scalar.activation(out=avec, in_=avec_st, func=AF.Copy)
            bqv_sb = dvec[:, 0:3]
            bkv_sb = dvec[:, 3:6]

            # xt (attention output) slabs + own h1 slab, zero-padded 34x34
            xt_sl = []
            for i in range(3):
                t = xtp.tile([128, HP, HP], BF16, name=f"xt{i}")
                nc.vector.memset(t, 0.0)
                xt_sl.append(t)
            h1_own = xtp.tile([128, HP, HP], BF16, name="h1own")
            nc.vector.memset(h1_own, 0.0)
            # conv1 accumulator (own 128 out channels, fp32)
            acc1 = xtp.tile([128, S], F32, name="acc1")

            # Semaphore warmers: absorb const-DMA + memset waits into each
            # engine's observed clock so later compute ops need <=1 wait.
            warm = const.tile([128, 1], F32)
            nc.vector.tensor_copy(warm, dvec[:, 0:1])
            warm2 = const.tile([128, 1], F32)
            nc.scalar.activation(out=warm2, in_=warm, func=AF.Copy)

            # k/q per branch (with biases added), uT tiles.
            # uT column layout per local head: even head -> [u0..u31, 1],
            # odd head -> [1, u0..u31]; so in the paired [66,512] y tile the
            # two softmax denominators land on adjacent rows 32, 33.
            k_sb = kq.tile([128, 3, S], BF16)
            q_sb = kq.tile([128, 3, S], BF16)
            uT = [kq.tile([128, 3, 4, 33], BF16, name=f"uT{t}") for t in range(8)]

            qsrc = [oth_sb[:, 0], x_sb, oth_sb[:, 1]]

            def proj_kq(i):
                k_ps = scps.tile([128, S], F32, name="kq_ps", tag="sc")
                for s in range(2):
                    for ks in range(4):
                        nc.tensor.matmul(
                            k_ps[:, 512 * s : 512 * (s + 1)],
                            lhsT=wk_sb[:, i, ks, :],
                            rhs=x_sb[:, ks, 512 * s : 512 * (s + 1)],
                            start=(ks == 0),
                            stop=(ks == 3),
                        )
                nc.vector.tensor_scalar_add(k_sb[:, i, :], k_ps, bkv_sb[:, i : i + 1])

                q_ps = scps.tile([128, S], F32, name="kq_ps2", tag="sc")
                for s in range(2):
                    for ks in range(4):
                        nc.tensor.matmul(
                            q_ps[:, 512 * s : 512 * (s + 1)],
                            lhsT=wq_sb[:, i, ks, :],
                            rhs=qsrc[i][:, ks, 512 * s : 512 * (s + 1)],
                            start=(ks == 0),
                            stop=(ks == 3),
                        )
                nc.vector.tensor_scalar_add(q_sb[:, i, :], q_ps, bqv_sb[:, i : i + 1])

            def proj_u(t):
                u_ps = yps.tile([128, 384], F32, name="u_ps", tag="y")
                for ks in range(4):
                    nc.tensor.matmul(
                        u_ps,
                        lhsT=x_sb[:, ks, 128 * t : 128 * (t + 1)],
                        rhs=wvo_sb[:, ks, :],
                        start=(ks == 0),
                        stop=(ks == 3),
                    )
                nc.vector.memset(uT[t][:, :, :, 32:33], 1.0)
                nc.vector.tensor_add(
                    uT[t][:, :, :, 0:32],
                    u_ps.rearrange("p (i h d) -> p i h d", i=3, h=4),
                    wobv_sb.rearrange("p (i h d) -> p i h d", i=3, h=4),
                )

            # ---- conv weights (after initial DMAs) -----------------------
            convw = ctx.enter_context(tc.tile_pool(name="convw", bufs=1))
            c1w_sb = [
                [convw.tile([128, 9, 128], BF16, name=f"c1w{i}_{k}") for k in range(4)]
                for i in range(3)
            ]
            c2w_sb = [convw.tile([128, 9, 128], BF16, name=f"c2w{k}") for k in range(4)]
            for k in range(4):
                nc.sync.dma_start(out=c1w_sb[0][k], in_=c1wT_d[0, k])
            for k in range(4):
                nc.sync.dma_start(out=c2w_sb[k], in_=c2wT_d[k])
            for i in range(1, 3):
                for k in range(4):
                    nc.sync.dma_start(out=c1w_sb[i][k], in_=c1wT_d[i, k])

            # gathered xt slabs per branch (own + 3 peers)
            NBR = 3 if DEBUG == 0 else 1
            xta = [xtp.tile([128, 4, HP, HP], BF16, name=f"xta{i}") for i in range(NBR)]
            h1_sl = [xtp.tile([128, HP, HP], BF16, name=f"h1{k}") for k in range(4 if DEBUG == 0 else 0)]

            # DRAM staging for collectives
            xt_stage = [dram.tile([128, HP * HP], BF16, name=f"xts{i}") for i in range(NBR)]
            xt_gath = [dram.tile([512, HP * HP], BF16, name=f"xtg{i}") for i in range(NBR)]
            h1_stage = [dram.tile([128, H1CH[a][1] * HP], BF16, name=f"h1s{a}") for a in range(2 if DEBUG == 0 else 0)]
            h1_gath = [dram.tile([512, H1CH[a][1] * HP], BF16, name=f"h1g{a}") for a in range(2 if DEBUG == 0 else 0)]

            rcp = ctx.enter_context(tc.tile_pool(name="rcp", bufs=4))

            # ------------- attention + conv1 building blocks --------------
            # conv1 partial for branch i, row-block n, k-slab k.  The four
            # k-slab sub-units of one (i, n) block share a cvps tile and one
            # PSUM accumulation group (interleaving matmuls to other banks
            # between sub-units is legal: group state is per zero-region).
            c1_state = {}

            def conv1_unit(i, n, k):
                r0, nr = C1ROWS[n]
                if k == 0:
                    c1_state["ps"] = cvps.tile([128, 512], F32, name="cv", tag="cv")
                ps = c1_state["ps"]
                for dy in range(3):
                    for dx in range(3):
                        nc.tensor.matmul(
                            ps[:, : nr * 32],
                            lhsT=c1w_sb[i][k][:, dy * 3 + dx, :],
                            rhs=xta[i][:, k, r0 + dy : r0 + dy + nr, dx : dx + 32],
                            start=(k == 0 and dy == 0 and dx == 0),
                            stop=(k == 3 and dy == 2 and dx == 2),
                        )
                if k == 3:
                    dst = acc1[:, r0 * 32 : (r0 + nr) * 32]
                    if i == 0:
                        nc.vector.tensor_copy(dst, ps[:, : nr * 32])
                    else:
                        nc.vector.tensor_add(dst, ps[:, : nr * 32], dst)

            def conv1_block(i, n):
                for k in range(4):
                    conv1_unit(i, n, k)

            def attention_pair(i, pr, fillers, fst, fire):
                """Head pair pr of branch i as one software-pipelined t-loop:
                scores(t)+exp(t) emitted one step ahead of y(t); filler units
                (conv1 sub-blocks / projections) keep PE busy while ACT
                grinds through the exps."""
                pts = {}

                def emit_sc(t):
                    for hh in range(2):
                        h = 2 * pr + hh
                        sc = scps.tile([128, S], F32, name="sc", tag="sc")
                        p0 = 32 * h
                        for s in range(2):
                            nc.tensor.matmul(
                                sc[:, 512 * s : 512 * (s + 1)],
                                lhsT=k_sb[p0 : p0 + 32, i, 128 * t : 128 * (t + 1)],
                                rhs=q_sb[p0 : p0 + 32, i, 512 * s : 512 * (s + 1)],
                                start=True,
                                stop=True,
                                tile_position=(p0, 0),
                            )
                        ptt = pt.tile([128, S], BF16, name="ptt")
                        nc.scalar.activation(
                            out=ptt, in_=sc, func=AF.Exp, scale=float(ISQD)
                        )
                        pts[(hh, t)] = ptt

                def emit_y(ytl, s, t):
                    for hh in range(2):
                        h = 2 * pr + hh
                        nc.tensor.matmul(
                            ytl[hh],
                            lhsT=uT[t][:, i, h, :],
                            rhs=pts[(hh, t)][:, 512 * s : 512 * (s + 1)],
                            start=(t == 0),
                            stop=(t == 7),
                        )

                def normalize(ytl, s):
                    for hh in range(2):
                        h = 2 * pr + hh
                        rc = rcp.tile([1, 512], F32, name="rc")
                        nc.vector.reciprocal(rc, ytl[hh][32:33, :])
                        rcd = dramw.tile([1, 512], F32, name="rcd", tag="rcd")
                        nc.gpsimd.dma_start(out=rcd, in_=rc)
                        rcb = rcp.tile([32, 512], F32, name="rcb")
                        nc.gpsimd.dma_start(out=rcb, in_=rcd[:].partition_broadcast(32))
                        if DEBUG == 2 and i == 0 and pr == 0:
                            idx = s * 2 + hh
                            nc.sync.dma_start(out=out_d[idx : idx + 1, 0:512], in_=rc)
                            dcp = rcp.tile([1, 512], F32, name="dcp")
                            nc.vector.tensor_copy(dcp, ytl[hh][32:33, :])
                            nc.sync.dma_start(
                                out=out_d[4 + idx : 5 + idx, 0:512], in_=dcp
                            )
                            nc.sync.dma_start(
                                out=out_d[8 + idx : 9 + idx, 0:512], in_=rcb[0:1, :]
                            )
                            nc.sync.dma_start(
                                out=out_d[12 + idx : 13 + idx, 0:512], in_=rcb[31:32, :]
                            )
                            nc.sync.dma_start(
                                out=out_d[16 + idx : 17 + idx, 0:512], in_=rcb[16:17, :]
                            )
                        p0 = 32 * h
                        nc.vector.tensor_mul(
                            xt_sl[i][p0 : p0 + 32, 1 + 16 * s : 17 + 16 * s, 1:33],
                            ytl[hh][0:32, :].rearrange("p (a b) -> p a b", b=32),
                            rcb.rearrange("p (a b) -> p a b", b=32),
                        )

                # s=0 accumulates inside the t-loop (PE trails ACT by one
                # exp); s=1 re-reads the kept pts tiles in a second pass.
                y0 = [yps.tile([33, 512], F32, name=f"y0_{hh}", tag="y") for hh in range(2)]
                for t in range(8):
                    emit_sc(t)
                    if t >= 1:
                        emit_y(y0, 0, t - 1)
                    if t in fire and fst[0] < len(fillers):
                        fillers[fst[0]]()
                        fst[0] += 1
                emit_y(y0, 0, 7)
                normalize(y0, 0)
                y1 = [yps.tile([33, 512], F32, name=f"y1_{hh}", tag="y") for hh in range(2)]
                for t in range(8):
                    emit_y(y1, 1, t)
                normalize(y1, 1)

            def ag_xt(i):
                nc.sync.dma_start(out=xt_stage[i][:], in_=xt_sl[i][:].rearrange("p a b -> p (a b)"))
                nc.gpsimd.collective_compute(
                    "AllGather",
                    mybir.AluOpType.bypass,
                    replica_groups=GROUPS,
                    ins=[xt_stage[i][:]],
                    outs=[xt_gath[i][:]],
                )
                for k in range(4):
                    nc.sync.dma_start(
                        out=xta[i][:, k, :, :].rearrange("p a b -> p (a b)"),
                        in_=xt_gath[i][128 * k : 128 * (k + 1), :],
                    )

            # ---------------- emission schedule ---------------------------
            # proj for branch 0 + uT first so attention can start early
            proj_kq(0)
            for t in range(8):
                proj_u(t)

            for i in range(3):
                # filler units to interleave during branch i's attention:
                # branch 0 gets the remaining projections, branches 1/2 get
                # the conv1 sub-blocks of branch i-1 (gathered by then).
                if i == 0:
                    fillers = [lambda: proj_kq(1), lambda: proj_kq(2)]
                    fire0, fire1 = {2, 5}, {2, 5}
                else:
                    fillers = [
                        (lambda n=n, k=k: conv1_unit(i - 1, n, k))
                        for n in range(3)
                        for k in range(4)
                    ]
                    # delay conv1 in pr0 so the xt AllGather has landed
                    fire0, fire1 = {4, 5, 6, 7}, {2, 3, 4, 5, 6, 7}
                fst = [0]
                attention_pair(i, 0, fillers, fst, fire0)
                attention_pair(i, 1, fillers, fst, fire1)
                while fst[0] < len(fillers):
                    fillers[fst[0]]()
                    fst[0] += 1
                ag_xt(i)
                if DEBUG >= 1 and i == 0:
                    nc.gpsimd.dma_start(
                        out=out_d[:].rearrange("p (a b) -> p a b", b=32),
                        in_=xt_sl[0][:, 1:33, 1:33],
                    )
                    break

            # ---- tail: conv1 for branch 2, h1, AG h1, conv2, out ---------
            conv1_block(2, 0) if DEBUG == 0 else None
            conv1_block(2, 1) if DEBUG == 0 else None

            def h1_relu(a):
                # relu(bn1(acc1)) for the valid rows covered by AG chunk a
                pr0, nr = H1CH[a]
                v0 = max(pr0 - 1, 0)          # first valid row in chunk
                v1 = min(pr0 + nr - 1, 32)    # end valid row (exclusive)
                nc.scalar.activation(
                    out=h1_own[:, 1 + v0 : 1 + v1, 1:33],
                    in_=acc1[:, 32 * v0 : 32 * v1].rearrange("p (a b) -> p a b", b=32),
                    func=AF.Relu,
                    bias=avec[:, 1:2],
                    scale=avec[:, 0:1],
                )

            def ag_h1(a):
                pr0, nr = H1CH[a]
                nc.sync.dma_start(
                    out=h1_stage[a][:],
                    in_=h1_own[:, pr0 : pr0 + nr, :].rearrange("p a b -> p (a b)"),
                )
                nc.gpsimd.collective_compute(
                    "AllGather",
                    mybir.AluOpType.bypass,
                    replica_groups=GROUPS,
                    ins=[h1_stage[a][:]],
                    outs=[h1_gath[a][:]],
                )
                for k in range(4):
                    nc.sync.dma_start(
                        out=h1_sl[k][:, pr0 : pr0 + nr, :].rearrange("p a b -> p (a b)"),
                        in_=h1_gath[a][128 * k : 128 * (k + 1), :],
                    )

            if DEBUG == 0:
                h1_relu(0)
                ag_h1(0)
                conv1_block(2, 2)
                h1_relu(1)
                ag_h1(1)

            oout = stg.tile([128, S], F32, name="oout", bufs=1)
            for n in range(2 if DEBUG == 0 else 0):
                ps2 = cvps.tile([128, 512], F32, name=f"cv2_{n}", tag="cv")
                for k in range(4):
                    for dy in range(3):
                        for dx in range(3):
                            nc.tensor.matmul(
                                ps2,
                                lhsT=c2w_sb[k][:, dy * 3 + dx, :],
                                rhs=h1_sl[k][
                                    :, 16 * n + dy : 16 * n + dy + 16, dx : dx + 32
                                ],
                                start=(k == 0 and dy == 0 and dx == 0),
                                stop=(k == 3 and dy == 2 and dx == 2),
                            )
                nc.scalar.activation(
                    out=oout[:, 512 * n : 512 * (n + 1)],
                    in_=ps2,
                    func=AF.Relu,
                    bias=avec[:, 3:4],
                    scale=avec[:, 2:3],
                )
                nc.sync.dma_start(
                    out=out_d[:, 512 * n : 512 * (n + 1)],
                    in_=oout[:, 512 * n : 512 * (n + 1)],
                )

    nc.finalize()
    return nc


def _f(x):
    return np.ascontiguousarray(x, dtype=np.float32)


def _bf(x):
    return np.ascontiguousarray(np.asarray(x, dtype=np.float32).astype(ml_dtypes.bfloat16))


def prepare_core_inputs(inp):
    """Build the 8 per-core input dicts from the full-problem inputs."""
    inp = {k: np.asarray(v, dtype=np.float64) for k, v in inp.items()}
    x = inp["x"].reshape(B, C, S)
    xp = inp["x_prev"].reshape(B, C, S)
    xn = inp["x_next"].reshape(B, C, S)

    bn1s_full = inp["bn1g"] / np.sqrt(inp["bn1v"] + EPS)
    bn1b_full = inp["bn1b"] - inp["bn1m"] * bn1s_full
    bn2s_full = inp["bn2g"] / np.sqrt(inp["bn2v"] + EPS)
    bn2b_full = inp["bn2b"] - inp["bn2m"] * bn2s_full

    per_g = []
    for g in range(4):
        sl = slice(128 * g, 128 * (g + 1))
        wqT = np.stack(
            [
                np.stack([inp["Wq"][i][sl, 128 * k : 128 * (k + 1)].T for k in range(4)])
                for i in range(3)
            ]
        )
        wkT = np.stack(
            [
                np.stack([inp["Wk"][i][sl, 128 * k : 128 * (k + 1)].T for k in range(4)])
                for i in range(3)
            ]
        )
        bqv = np.stack([inp["bq"][i][sl] for i in range(3)], axis=1)
        bkv = np.stack([inp["bk"][i][sl] for i in range(3)], axis=1)

        att_s = np.stack(
            [inp["bng"][i][sl] / np.sqrt(inp["bnv"][i][sl] + EPS) for i in range(3)]
        )  # (3,128)
        xtb = np.stack(
            [
                inp["bnb"][i][sl] + (inp["bo"][i][sl] - inp["bnm"][i][sl]) * att_s[i]
                for i in range(3)
            ]
        )  # (3,128)  full BN bias for this slab, folded into wobv below

        wvo_rows = []
        wobv_row = []
        for i in range(3):
            for hl in range(4):
                hg = 4 * g + hl
                wv_h = inp["Wv"][i][32 * hg : 32 * (hg + 1), :]  # (32, 512)
                bv_h = inp["bv"][i][32 * hg : 32 * (hg + 1)]
                wo_h = inp["Wo"][i, hg]  # (32, 32)
                sc = att_s[i][32 * hl : 32 * (hl + 1)]  # (32,)
                wvo_rows.append(sc[:, None] * (wo_h @ wv_h))
                # fold the full BN/bias offset in: y/den then equals bn'd out
                wobv_row.append(sc * (wo_h @ bv_h) + xtb[i][32 * hl : 32 * (hl + 1)])
        wvo_all = np.concatenate(wvo_rows, axis=0)  # (384, 512)
        wobv = np.concatenate(wobv_row)[None, :]  # (1, 384)
        wvoT = np.stack([wvo_all[:, 128 * k : 128 * (k + 1)].T for k in range(4)])

        # conv1 weights: own 128 out-ch over all 1536 in-ch
        c1wT = np.stack(
            [
                np.stack(
                    [
                        inp["c1w"][sl, 512 * i + 128 * k : 512 * i + 128 * (k + 1)]
                        .transpose(1, 2, 3, 0)
                        .reshape(128, 9, 128)
                        for k in range(4)
                    ]
                )
                for i in range(3)
            ]
        )
        c2wT = np.stack(
            [
                inp["c2w"][sl, 128 * k : 128 * (k + 1)]
                .transpose(1, 2, 3, 0)
                .reshape(128, 9, 128)
                for k in range(4)
            ]
        )
        avec = np.stack(
            [bn1s_full[sl], bn1b_full[sl], bn2s_full[sl], bn2b_full[sl]], axis=1
        )  # (128, 4)

        per_g.append(
            dict(
                wqT=_bf(wqT), wkT=_bf(wkT), wvoT=_bf(wvoT),
                wobv=_f(wobv), c1wT=_bf(c1wT), c2wT=_bf(c2wT),
                dvec=_f(np.concatenate([bqv, bkv], axis=1)),
                avec=_f(avec),
            )
        )

    in_maps = []
    for c in range(NCORES):
        b, g = c // 4, c % 4
        d = dict(per_g[g])
        d["x4"] = _bf(x[b].reshape(4, 128, S))
        d["oth"] = _bf(np.stack([xn[b].reshape(4, 128, S), xp[b].reshape(4, 128, S)]))
        in_maps.append(d)
    return in_maps


_NC_CACHE = {}


def get_nc():
    if "nc" not in _NC_CACHE:
        _NC_CACHE["nc"] = build_nc()
    return _NC_CACHE["nc"]


def assemble(results):
    out = np.zeros((B, C, H, W), dtype=np.float32)
    for c in range(NCORES):
        b, g = c // 4, c % 4
        out[b, 128 * g : 128 * (g + 1)] = results[c]["out"].reshape(128, H, W)
    return out


def kernel(**inputs):
    nc = get_nc()
    in_maps = prepare_core_inputs(inputs)
    res = run_bass_kernel_spmd(nc, in_maps, list(range(NCORES)))
    return assemble(res.results)


# revision 25
# speedup vs baseline: 1.1816x; 1.0617x over previous
"""CSAEncoder Trainium2 kernel: 3-branch cross-attention + concat DoubleConv.

Sharding (8 cores): 2 batch groups x 4 tensor ranks.
Core c: batch b = c // 4, rank g = c % 4.
  - Attention: core computes heads [4g, 4g+4) of all 3 branches for batch b
    (a contiguous 128-channel slab of each branch's output, written as a
    zero-padded 34x34 bf16 slab).
  - conv1: per-branch LOCAL partial sums (own 128 input channels of that
    branch, ALL 512 output channels), staged to DRAM in bf16 and combined
    with a per-branch ReduceScatter(add) within the 4-core batch group, so
    each rank accumulates its own 128 output channels into acc1.  The
    branch-i partial conv is interleaved into branch-(i+1) attention as PE
    filler (depends only on local data - no collective wait), and the
    branch-i ReduceScatter itself overlaps branch-(i+1) compute.
  - h1 = relu(bn1(acc1)) for the own 128 channels; conv2 partial (own 128
    h1 channels, all 512 outputs) + one final ReduceScatter; BN2+relu.
Host assembles the full (2, 512, 32, 32) output from the 8 per-core slabs.

Softmax: the y matmul appends a ones column per head (u | 1) so row 32 of
each [33,512] PSUM y tile is the softmax denominator; the BN bias is folded
into the v/o projection bias on the host so xt = y * (1/den) exactly.  The
reciprocal is batched two denominators at a time on DVE and broadcast to 32
partitions via a DRAM round-trip on the (otherwise idle) GpSimd queue.

A tiny barrier AllGather right after launch absorbs core launch skew while
the input DMAs stream, so the later collectives see small peer skew.
"""

import os
import sys

import ml_dtypes
import numpy as np

for _p in ("/opt/trn_rl_repo",):
    if _p not in sys.path and os.path.isdir(_p):
        sys.path.insert(0, _p)

import concourse.bass as bass
import concourse.mybir as mybir
import concourse.tile as tile
from concourse import bacc
from concourse.bass_utils import run_bass_kernel_spmd

F32 = mybir.dt.float32
BF16 = mybir.dt.bfloat16
AF = mybir.ActivationFunctionType

B, C, H, W, HEADS = 2, 512, 32, 32, 16
D = C // HEADS            # 32
S = H * W                 # 1024
EPS = 1e-5
ISQD = 1.0 / np.sqrt(D)   # folded into the exp activation
NCORES = 8
GROUPS = [[0, 1, 2, 3], [4, 5, 6, 7]]
HP = W + 2                # padded row stride (34)


def build_nc():
    nc = bacc.Bacc(None, target_bir_lowering=False)

    # ---- per-core external inputs -------------------------------------
    x4_d = nc.declare_dram_parameter("x4", [4, 128, S], BF16, isOutput=False)
    oth_d = nc.declare_dram_parameter("oth", [2, 4, 128, S], BF16, isOutput=False)
    wqT_d = nc.declare_dram_parameter("wqT", [3, 4, 128, 128], BF16, isOutput=False)
    wkT_d = nc.declare_dram_parameter("wkT", [3, 4, 128, 128], BF16, isOutput=False)
    wvoT_d = nc.declare_dram_parameter("wvoT", [4, 128, 384], BF16, isOutput=False)
    dvec_d = nc.declare_dram_parameter("dvec", [128, 6], F32, isOutput=False)
    wobv_d = nc.declare_dram_parameter("wobv", [1, 384], F32, isOutput=False)
    c1wT_d = nc.declare_dram_parameter("c1wT", [3, 4, 128, 9, 128], BF16, isOutput=False)
    c2wT_d = nc.declare_dram_parameter("c2wT", [4, 128, 9, 128], BF16, isOutput=False)
    avec_d = nc.declare_dram_parameter("avec", [128, 4], F32, isOutput=False)
    out_d = nc.declare_dram_parameter("out", [128, S], F32, isOutput=True)

    with tile.TileContext(nc) as tc:
        import contextlib

        ctx = contextlib.ExitStack()
        with ctx:
            const = ctx.enter_context(tc.tile_pool(name="const", bufs=1))
            kq = ctx.enter_context(tc.tile_pool(name="kq", bufs=1))
            xtp = ctx.enter_context(tc.tile_pool(name="xtp", bufs=1))
            stg = ctx.enter_context(tc.tile_pool(name="stg", bufs=6))
            scps = ctx.enter_context(tc.tile_pool(name="scps", bufs=2, space="PSUM"))
            yps = ctx.enter_context(tc.tile_pool(name="yps", bufs=3, space="PSUM"))
            cvps = ctx.enter_context(tc.tile_pool(name="cvps", bufs=1, space="PSUM"))
            dram = ctx.enter_context(tc.tile_pool(name="dram", bufs=1, space="DRAM"))
            dramw = ctx.enter_context(tc.tile_pool(name="dramw", bufs=4, space="DRAM"))
            pt = ctx.enter_context(tc.tile_pool(name="pt", bufs=20))
            rcp = ctx.enter_context(tc.tile_pool(name="rcp", bufs=4))

            # ---- barrier collective to absorb launch skew ----------------
            bar_in = dram.tile([1, 4], F32, name="barin")
            bar_out = dram.tile([4, 4], F32, name="barout")
            nc.sync.dma_start(out=bar_in[:], in_=dvec_d[0:1, 0:4])
            nc.gpsimd.collective_compute(
                "AllGather",
                mybir.AluOpType.bypass,
                replica_groups=GROUPS,
                ins=[bar_in[:]],
                outs=[bar_out[:]],
            )

            # ---- first-needed DMAs (weights for proj 0, x, q-source) -----
            wq_sb = const.tile([128, 3, 4, 128], BF16)
            wk_sb = const.tile([128, 3, 4, 128], BF16)
            x_sb = const.tile([128, 4, S], BF16)
            oth_sb = const.tile([128, 2, 4, S], BF16)
            wvo_sb = const.tile([128, 4, 384], BF16)
            for ks in range(4):
                nc.sync.dma_start(out=wk_sb[:, 0, ks, :], in_=wkT_d[0, ks])
                nc.sync.dma_start(out=wq_sb[:, 0, ks, :], in_=wqT_d[0, ks])
            for ks in range(4):
                nc.sync.dma_start(out=x_sb[:, ks, :], in_=x4_d[ks])
            for ks in range(4):
                nc.sync.dma_start(out=oth_sb[:, 0, ks, :], in_=oth_d[0, ks])
            for ks in range(4):
                nc.sync.dma_start(out=wvo_sb[:, ks, :], in_=wvoT_d[ks])
            for i in range(1, 3):
                for ks in range(4):
                    nc.sync.dma_start(out=wk_sb[:, i, ks, :], in_=wkT_d[i, ks])
                    nc.sync.dma_start(out=wq_sb[:, i, ks, :], in_=wqT_d[i, ks])
            for ks in range(4):
                nc.sync.dma_start(out=oth_sb[:, 1, ks, :], in_=oth_d[1, ks])

            # Small consts: DMA to staging, then re-own on the consuming
            # engine (DVE / ACT) so consumers need no cross-engine const wait.
            dvec_st = const.tile([128, 6], F32)
            nc.gpsimd.dma_start(out=dvec_st, in_=dvec_d[:])
            wobv_st = const.tile([128, 384], F32)
            nc.gpsimd.dma_start(out=wobv_st, in_=wobv_d[:].partition_broadcast(128))
            avec_st = const.tile([128, 4], F32)
            nc.gpsimd.dma_start(out=avec_st, in_=avec_d[:])
            dvec = const.tile([128, 6], F32)
            nc.vector.tensor_copy(dvec, dvec_st)
            wobv_sb = const.tile([128, 384], F32)
            nc.vector.tensor_copy(wobv_sb, wobv_st)
            avec = const.tile([128, 4], F32)
            nc.scalar.activation(out=avec, in_=avec_st, func=AF.Copy)
            bqv_sb = dvec[:, 0:3]
            bkv_sb = dvec[:, 3:6]

            # xt (attention output) slabs + own h1 slab, zero-padded 34x34
            xt_sl = []
            for i in range(3):
                t = xtp.tile([128, HP, HP], BF16, name=f"xt{i}")
                nc.vector.memset(t, 0.0)
                xt_sl.append(t)
            h1_own = xtp.tile([128, HP, HP], BF16, name="h1own")
            nc.vector.memset(h1_own, 0.0)
            # conv1 accumulator (own 128 out channels, fp32)
            acc1 = xtp.tile([128, S], F32, name="acc1")

            # Semaphore warmers: absorb const-DMA + memset waits into each
            # engine's observed clock so later compute ops need <=1 wait.
            warm = const.tile([128, 1], F32)
            nc.vector.tensor_copy(warm, dvec[:, 0:1])
            warm2 = const.tile([128, 1], F32)
            nc.scalar.activation(out=warm2, in_=warm, func=AF.Copy)

            # k/q per branch (with biases added), uT tiles ([u | 1] cols)
            k_sb = kq.tile([128, 3, S], BF16)
            q_sb = kq.tile([128, 3, S], BF16)
            uT = [kq.tile([128, 3, 4, 33], BF16, name=f"uT{t}") for t in range(8)]

            qsrc = [oth_sb[:, 0], x_sb, oth_sb[:, 1]]

            def proj_kq(i):
                k_ps = scps.tile([128, S], F32, name="kq_ps", tag="sc")
                for s in range(2):
                    for ks in range(4):
                        nc.tensor.matmul(
                            k_ps[:, 512 * s : 512 * (s + 1)],
                            lhsT=wk_sb[:, i, ks, :],
                            rhs=x_sb[:, ks, 512 * s : 512 * (s + 1)],
                            start=(ks == 0),
                            stop=(ks == 3),
                        )
                nc.vector.tensor_scalar_add(k_sb[:, i, :], k_ps, bkv_sb[:, i : i + 1])

                q_ps = scps.tile([128, S], F32, name="kq_ps2", tag="sc")
                for s in range(2):
                    for ks in range(4):
                        nc.tensor.matmul(
                            q_ps[:, 512 * s : 512 * (s + 1)],
                            lhsT=wq_sb[:, i, ks, :],
                            rhs=qsrc[i][:, ks, 512 * s : 512 * (s + 1)],
                            start=(ks == 0),
                            stop=(ks == 3),
                        )
                nc.vector.tensor_scalar_add(q_sb[:, i, :], q_ps, bqv_sb[:, i : i + 1])

            def proj_u(t):
                u_ps = yps.tile([128, 384], F32, name="u_ps", tag="y")
                for ks in range(4):
                    nc.tensor.matmul(
                        u_ps,
                        lhsT=x_sb[:, ks, 128 * t : 128 * (t + 1)],
                        rhs=wvo_sb[:, ks, :],
                        start=(ks == 0),
                        stop=(ks == 3),
                    )
                nc.vector.memset(uT[t][:, :, :, 32:33], 1.0)
                nc.vector.tensor_add(
                    uT[t][:, :, :, 0:32],
                    u_ps.rearrange("p (i h d) -> p i h d", i=3, h=4),
                    wobv_sb.rearrange("p (i h d) -> p i h d", i=3, h=4),
                )

            # ---- conv weights (after initial DMAs) -----------------------
            convw = ctx.enter_context(tc.tile_pool(name="convw", bufs=1))
            c1w_sb = [
                [convw.tile([128, 9, 128], BF16, name=f"c1w{i}_{m}") for m in range(4)]
                for i in range(3)
            ]
            c2w_sb = [convw.tile([128, 9, 128], BF16, name=f"c2w{m}") for m in range(4)]
            for m in range(4):
                nc.sync.dma_start(out=c1w_sb[0][m], in_=c1wT_d[0, m])
            for m in range(4):
                nc.sync.dma_start(out=c2w_sb[m], in_=c2wT_d[m])
            for i in range(1, 3):
                for m in range(4):
                    nc.sync.dma_start(out=c1w_sb[i][m], in_=c1wT_d[i, m])

            # DRAM staging for the ReduceScatters (bf16 partials)
            part1_d = [dram.tile([512, S], BF16, name=f"p1_{i}") for i in range(3)]
            rs1_d = [dram.tile([128, S], BF16, name=f"r1_{i}") for i in range(3)]
            part2_d = dram.tile([512, S], BF16, name="p2")
            rs2_d = dram.tile([128, S], BF16, name="r2")

            # ------------- conv partial building blocks -------------------
            def conv1_unit(i, m, n):
                """Local conv1 partial: branch i's own 128 in-ch, out m-tile,
                spatial half n; result staged to DRAM (bf16) for the RS."""
                ps = cvps.tile([128, 512], F32, name="cv", tag="cv")
                for dy in range(3):
                    for dx in range(3):
                        nc.tensor.matmul(
                            ps,
                            lhsT=c1w_sb[i][m][:, dy * 3 + dx, :],
                            rhs=xt_sl[i][:, 16 * n + dy : 16 * n + dy + 16, dx : dx + 32],
                            start=(dy == 0 and dx == 0),
                            stop=(dy == 2 and dx == 2),
                        )
                st = stg.tile([128, 512], BF16, name="c1st")
                nc.vector.tensor_copy(st, ps)
                nc.sync.dma_start(
                    out=part1_d[i][128 * m : 128 * (m + 1), 512 * n : 512 * (n + 1)],
                    in_=st,
                )

            def rs1(i):
                nc.gpsimd.collective_compute(
                    "ReduceScatter",
                    mybir.AluOpType.add,
                    replica_groups=GROUPS,
                    ins=[part1_d[i][:]],
                    outs=[rs1_d[i][:]],
                )
                rsb = stg.tile([128, S], BF16, name="rsb")
                nc.sync.dma_start(out=rsb, in_=rs1_d[i][:])
                if i == 0:
                    nc.vector.tensor_copy(acc1, rsb)
                else:
                    nc.vector.tensor_add(acc1, rsb, acc1)

            def attention_pair(i, pr, fillers, fst, fire):
                """Head pair pr of branch i as one software-pipelined t-loop:
                scores(t)+exp(t) emitted one step ahead of y(t); filler units
                (conv1 partial blocks / projections) keep PE busy while ACT
                grinds through the exps."""
                pts = {}

                def emit_sc(t):
                    for hh in range(2):
                        h = 2 * pr + hh
                        sc = scps.tile([128, S], F32, name="sc", tag="sc")
                        p0 = 32 * h
                        for s in range(2):
                            nc.tensor.matmul(
                                sc[:, 512 * s : 512 * (s + 1)],
                                lhsT=k_sb[p0 : p0 + 32, i, 128 * t : 128 * (t + 1)],
                                rhs=q_sb[p0 : p0 + 32, i, 512 * s : 512 * (s + 1)],
                                start=True,
                                stop=True,
                                tile_position=(p0, 0),
                            )
                        ptt = pt.tile([128, S], BF16, name="ptt")
                        nc.scalar.activation(
                            out=ptt, in_=sc, func=AF.Exp, scale=float(ISQD)
                        )
                        pts[(hh, t)] = ptt

                def emit_y(ytl, s, t):
                    for hh in range(2):
                        h = 2 * pr + hh
                        nc.tensor.matmul(
                            ytl[hh],
                            lhsT=uT[t][:, i, h, :],
                            rhs=pts[(hh, t)][:, 512 * s : 512 * (s + 1)],
                            start=(t == 0),
                            stop=(t == 7),
                        )

                def normalize(ytl, s):
                    # recip_approx_fast requires SBUF partition-0 input and
                    # partition_broadcast a partition-0 source (both HW-
                    # verified), so copy each denominator row down first.
                    for hh in range(2):
                        h = 2 * pr + hh
                        dh = rcp.tile([1, 512], F32, name="dh")
                        nc.vector.tensor_copy(dh, ytl[hh][32:33, :])
                        rch = rcp.tile([1, 512], F32, name="rch")
                        nc.vector.reciprocal_approx_fast(out=rch, in_=dh)
                        rcb = rcp.tile([32, 512], F32, name="rcb")
                        nc.gpsimd.partition_broadcast(rcb, rch)
                        p0 = 32 * h
                        nc.vector.tensor_mul(
                            xt_sl[i][p0 : p0 + 32, 1 + 16 * s : 17 + 16 * s, 1:33],
                            ytl[hh][0:32, :].rearrange("p (a b) -> p a b", b=32),
                            rcb.rearrange("p (a b) -> p a b", b=32),
                        )

                # s=0 accumulates inside the t-loop (PE trails ACT by one
                # exp); s=1 re-reads the kept pts tiles in a second pass.
                y0 = [yps.tile([33, 512], F32, name=f"y0_{hh}", tag="y") for hh in range(2)]
                for t in range(8):
                    emit_sc(t)
                    if t >= 1:
                        emit_y(y0, 0, t - 1)
                    if t in fire and fst[0] < len(fillers):
                        fillers[fst[0]]()
                        fst[0] += 1
                emit_y(y0, 0, 7)
                normalize(y0, 0)
                y1 = [yps.tile([33, 512], F32, name=f"y1_{hh}", tag="y") for hh in range(2)]
                for t in range(8):
                    emit_y(y1, 1, t)
                normalize(y1, 1)

            # ---------------- emission schedule ---------------------------
            # proj for branch 0 + uT first so attention can start early
            proj_kq(0)
            for t in range(8):
                proj_u(t)

            for i in range(3):
                # filler units during branch i's attention: branch 0 gets the
                # remaining projections, branches 1/2 get the LOCAL conv1
                # partial blocks of branch i-1 (no collective dependency).
                if i == 0:
                    fillers = [lambda: proj_kq(1), lambda: proj_kq(2)]
                    fire0, fire1 = {2, 5}, {2, 5}
                else:
                    fillers = [
                        (lambda m=m, n=n: conv1_unit(i - 1, m, n))
                        for n in range(2)
                        for m in range(4)
                    ]
                    fire0, fire1 = {2, 3, 4, 5}, {2, 3, 4, 5}
                fst = [0]
                attention_pair(i, 0, fillers, fst, fire0)
                attention_pair(i, 1, fillers, fst, fire1)
                while fst[0] < len(fillers):
                    fillers[fst[0]]()
                    fst[0] += 1
                if i > 0:
                    rs1(i - 1)

            # ---- tail: conv1 partial for branch 2, RS, h1, conv2, RS -----
            for n in range(2):
                for m in range(4):
                    conv1_unit(2, m, n)
            rs1(2)

            # h1 = relu(bn1(acc1)) into the padded own slab
            nc.scalar.activation(
                out=h1_own[:, 1:33, 1:33],
                in_=acc1.rearrange("p (a b) -> p a b", b=32),
                func=AF.Relu,
                bias=avec[:, 1:2],
                scale=avec[:, 0:1],
            )

            # conv2 partial: own 128 h1 channels -> all 512 outputs
            for n in range(2):
                for m in range(4):
                    ps2 = cvps.tile([128, 512], F32, name="cv2", tag="cv")
                    for dy in range(3):
                        for dx in range(3):
                            nc.tensor.matmul(
                                ps2,
                                lhsT=c2w_sb[m][:, dy * 3 + dx, :],
                                rhs=h1_own[
                                    :, 16 * n + dy : 16 * n + dy + 16, dx : dx + 32
                                ],
                                start=(dy == 0 and dx == 0),
                                stop=(dy == 2 and dx == 2),
                            )
                    st = stg.tile([128, 512], BF16, name="c2st")
                    nc.vector.tensor_copy(st, ps2)
                    nc.sync.dma_start(
                        out=part2_d[128 * m : 128 * (m + 1), 512 * n : 512 * (n + 1)],
                        in_=st,
                    )
            nc.gpsimd.collective_compute(
                "ReduceScatter",
                mybir.AluOpType.add,
                replica_groups=GROUPS,
                ins=[part2_d[:]],
                outs=[rs2_d[:]],
            )
            rs2b = stg.tile([128, S], BF16, name="rs2b")
            nc.sync.dma_start(out=rs2b, in_=rs2_d[:])
            oout = stg.tile([128, S], F32, name="oout", bufs=1)
            nc.scalar.activation(
                out=oout,
                in_=rs2b,
                func=AF.Relu,
                bias=avec[:, 3:4],
                scale=avec[:, 2:3],
            )
            nc.sync.dma_start(out=out_d[:], in_=oout)

    nc.finalize()
    return nc


def _f(x):
    return np.ascontiguousarray(x, dtype=np.float32)


def _bf(x):
    return np.ascontiguousarray(np.asarray(x, dtype=np.float32).astype(ml_dtypes.bfloat16))


def prepare_core_inputs(inp):
    """Build the 8 per-core input dicts from the full-problem inputs."""
    inp = {k: np.asarray(v, dtype=np.float64) for k, v in inp.items()}
    x = inp["x"].reshape(B, C, S)
    xp = inp["x_prev"].reshape(B, C, S)
    xn = inp["x_next"].reshape(B, C, S)

    bn1s_full = inp["bn1g"] / np.sqrt(inp["bn1v"] + EPS)
    bn1b_full = inp["bn1b"] - inp["bn1m"] * bn1s_full
    bn2s_full = inp["bn2g"] / np.sqrt(inp["bn2v"] + EPS)
    bn2b_full = inp["bn2b"] - inp["bn2m"] * bn2s_full

    per_g = []
    for g in range(4):
        sl = slice(128 * g, 128 * (g + 1))
        wqT = np.stack(
            [
                np.stack([inp["Wq"][i][sl, 128 * k : 128 * (k + 1)].T for k in range(4)])
                for i in range(3)
            ]
        )
        wkT = np.stack(
            [
                np.stack([inp["Wk"][i][sl, 128 * k : 128 * (k + 1)].T for k in range(4)])
                for i in range(3)
            ]
        )
        bqv = np.stack([inp["bq"][i][sl] for i in range(3)], axis=1)
        bkv = np.stack([inp["bk"][i][sl] for i in range(3)], axis=1)

        att_s = np.stack(
            [inp["bng"][i][sl] / np.sqrt(inp["bnv"][i][sl] + EPS) for i in range(3)]
        )  # (3,128)
        xtb = np.stack(
            [
                inp["bnb"][i][sl] + (inp["bo"][i][sl] - inp["bnm"][i][sl]) * att_s[i]
                for i in range(3)
            ]
        )  # (3,128)  full BN bias for this slab, folded into wobv below

        wvo_rows = []
        wobv_row = []
        for i in range(3):
            for hl in range(4):
                hg = 4 * g + hl
                wv_h = inp["Wv"][i][32 * hg : 32 * (hg + 1), :]  # (32, 512)
                bv_h = inp["bv"][i][32 * hg : 32 * (hg + 1)]
                wo_h = inp["Wo"][i, hg]  # (32, 32)
                sc = att_s[i][32 * hl : 32 * (hl + 1)]  # (32,)
                wvo_rows.append(sc[:, None] * (wo_h @ wv_h))
                # fold the full BN/bias offset in: y/den then equals bn'd out
                wobv_row.append(sc * (wo_h @ bv_h) + xtb[i][32 * hl : 32 * (hl + 1)])
        wvo_all = np.concatenate(wvo_rows, axis=0)  # (384, 512)
        wobv = np.concatenate(wobv_row)[None, :]  # (1, 384)
        wvoT = np.stack([wvo_all[:, 128 * k : 128 * (k + 1)].T for k in range(4)])

        # conv1 weights: own 128 in-ch (per branch) -> all 512 out (4 m-tiles)
        c1wT = np.stack(
            [
                np.stack(
                    [
                        inp["c1w"][
                            128 * m : 128 * (m + 1),
                            512 * i + 128 * g : 512 * i + 128 * (g + 1),
                        ]
                        .transpose(1, 2, 3, 0)
                        .reshape(128, 9, 128)
                        for m in range(4)
                    ]
                )
                for i in range(3)
            ]
        )
        # conv2 weights: own 128 h1-ch -> all 512 out (4 m-tiles)
        c2wT = np.stack(
            [
                inp["c2w"][128 * m : 128 * (m + 1), sl]
                .transpose(1, 2, 3, 0)
                .reshape(128, 9, 128)
                for m in range(4)
            ]
        )
        avec = np.stack(
            [bn1s_full[sl], bn1b_full[sl], bn2s_full[sl], bn2b_full[sl]], axis=1
        )  # (128, 4)

        per_g.append(
            dict(
                wqT=_bf(wqT), wkT=_bf(wkT), wvoT=_bf(wvoT),
                wobv=_f(wobv), c1wT=_bf(c1wT), c2wT=_bf(c2wT),
                dvec=_f(np.concatenate([bqv, bkv], axis=1)),
                avec=_f(avec),
            )
        )

    in_maps = []
    for c in range(NCORES):
        b, g = c // 4, c % 4
        d = dict(per_g[g])
        d["x4"] = _bf(x[b].reshape(4, 128, S))
        d["oth"] = _bf(np.stack([xn[b].reshape(4, 128, S), xp[b].reshape(4, 128, S)]))
        in_maps.append(d)
    return in_maps


_NC_CACHE = {}


def get_nc():
    if "nc" not in _NC_CACHE:
        _NC_CACHE["nc"] = build_nc()
    return _NC_CACHE["nc"]


def assemble(results):
    out = np.zeros((B, C, H, W), dtype=np.float32)
    for c in range(NCORES):
        b, g = c // 4, c % 4
        out[b, 128 * g : 128 * (g + 1)] = results[c]["out"].reshape(128, H, W)
    return out


def kernel(**inputs):
    nc = get_nc()
    in_maps = prepare_core_inputs(inputs)
    res = run_bass_kernel_spmd(nc, in_maps, list(range(NCORES)))
    return assemble(res.results)


# revision 30
# speedup vs baseline: 1.2422x; 1.0513x over previous
"""CSAEncoder Trainium2 kernel: 3-branch cross-attention + concat DoubleConv.

Sharding (8 cores): 2 batch groups x 4 tensor ranks.
Core c: batch b = c // 4, rank g = c % 4.
  - Attention: core computes heads [4g, 4g+4) of all 3 branches for batch b
    (a contiguous 128-channel slab of each branch's output, written as a
    zero-padded 34x34 bf16 slab).
  - conv1: per-branch LOCAL partial sums (own 128 input channels of that
    branch, ALL 512 output channels), staged to DRAM in bf16 and combined
    with a per-branch ReduceScatter(add) within the 4-core batch group, so
    each rank accumulates its own 128 output channels into acc1.  The
    branch-i partial conv is interleaved into branch-(i+1) attention as PE
    filler (depends only on local data - no collective wait), and the
    branch-i ReduceScatter itself overlaps branch-(i+1) compute.
  - h1 = relu(bn1(acc1)) for the own 128 channels; conv2 partial (own 128
    h1 channels, all 512 outputs) + one final ReduceScatter; BN2+relu.
Host assembles the full (2, 512, 32, 32) output from the 8 per-core slabs.

Softmax: the y matmul appends a ones column per head (u | 1) so row 32 of
each [33,512] PSUM y tile is the softmax denominator; the BN bias is folded
into the v/o projection bias on the host so xt = y * (1/den) exactly.  The
reciprocal is batched two denominators at a time on DVE and broadcast to 32
partitions via a DRAM round-trip on the (otherwise idle) GpSimd queue.

A tiny barrier AllGather right after launch absorbs core launch skew while
the input DMAs stream, so the later collectives see small peer skew.
"""

import os
import sys

import ml_dtypes
import numpy as np

for _p in ("/opt/trn_rl_repo",):
    if _p not in sys.path and os.path.isdir(_p):
        sys.path.insert(0, _p)

import concourse.bass as bass
import concourse.mybir as mybir
import concourse.tile as tile
from concourse import bacc
from concourse.bass_utils import run_bass_kernel_spmd

F32 = mybir.dt.float32
BF16 = mybir.dt.bfloat16
AF = mybir.ActivationFunctionType

B, C, H, W, HEADS = 2, 512, 32, 32, 16
D = C // HEADS            # 32
S = H * W                 # 1024
EPS = 1e-5
ISQD = 1.0 / np.sqrt(D)   # folded into the exp activation
NCORES = 8
GROUPS = [[0, 1, 2, 3], [4, 5, 6, 7]]
HP = W + 2                # padded row stride (34)


def build_nc():
    nc = bacc.Bacc(None, target_bir_lowering=False)

    # ---- per-core external inputs -------------------------------------
    x4_d = nc.declare_dram_parameter("x4", [4, 128, S], BF16, isOutput=False)
    oth_d = nc.declare_dram_parameter("oth", [2, 4, 128, S], BF16, isOutput=False)
    wqT_d = nc.declare_dram_parameter("wqT", [3, 4, 128, 128], BF16, isOutput=False)
    wkT_d = nc.declare_dram_parameter("wkT", [3, 4, 128, 128], BF16, isOutput=False)
    wvoT_d = nc.declare_dram_parameter("wvoT", [4, 128, 384], BF16, isOutput=False)
    dvec_d = nc.declare_dram_parameter("dvec", [128, 6], F32, isOutput=False)
    wobv_d = nc.declare_dram_parameter("wobv", [1, 384], F32, isOutput=False)
    c1wT_d = nc.declare_dram_parameter("c1wT", [3, 4, 128, 9, 128], BF16, isOutput=False)
    c2wT_d = nc.declare_dram_parameter("c2wT", [4, 128, 9, 128], BF16, isOutput=False)
    avec_d = nc.declare_dram_parameter("avec", [128, 4], F32, isOutput=False)
    out_d = nc.declare_dram_parameter("out", [128, S], F32, isOutput=True)

    with tile.TileContext(nc) as tc:
        import contextlib

        ctx = contextlib.ExitStack()
        with ctx:
            const = ctx.enter_context(tc.tile_pool(name="const", bufs=1))
            kq = ctx.enter_context(tc.tile_pool(name="kq", bufs=1))
            xtp = ctx.enter_context(tc.tile_pool(name="xtp", bufs=1))
            stg = ctx.enter_context(tc.tile_pool(name="stg", bufs=6))
            scps = ctx.enter_context(tc.tile_pool(name="scps", bufs=2, space="PSUM"))
            yps = ctx.enter_context(tc.tile_pool(name="yps", bufs=3, space="PSUM"))
            cvps = ctx.enter_context(tc.tile_pool(name="cvps", bufs=1, space="PSUM"))
            dram = ctx.enter_context(tc.tile_pool(name="dram", bufs=1, space="DRAM"))
            dramw = ctx.enter_context(tc.tile_pool(name="dramw", bufs=4, space="DRAM"))
            pt = ctx.enter_context(tc.tile_pool(name="pt", bufs=17))
            rcp = ctx.enter_context(tc.tile_pool(name="rcp", bufs=4))

            # ---- barrier collective to absorb launch skew ----------------
            bar_in = dram.tile([1, 4], F32, name="barin")
            bar_out = dram.tile([4, 4], F32, name="barout")
            nc.sync.dma_start(out=bar_in[:], in_=dvec_d[0:1, 0:4])
            nc.gpsimd.collective_compute(
                "AllGather",
                mybir.AluOpType.bypass,
                replica_groups=GROUPS,
                ins=[bar_in[:]],
                outs=[bar_out[:]],
            )

            # ---- first-needed DMAs (weights for proj 0, x, q-source) -----
            wq_sb = const.tile([128, 3, 4, 128], BF16)
            wk_sb = const.tile([128, 3, 4, 128], BF16)
            x_sb = const.tile([128, 4, S], BF16)
            oth_sb = const.tile([128, 2, 4, S], BF16)
            wvo_sb = const.tile([128, 4, 384], BF16)
            for ks in range(4):
                nc.sync.dma_start(out=wk_sb[:, 0, ks, :], in_=wkT_d[0, ks])
                nc.sync.dma_start(out=wq_sb[:, 0, ks, :], in_=wqT_d[0, ks])
            for ks in range(4):
                nc.sync.dma_start(out=x_sb[:, ks, :], in_=x4_d[ks])
            for ks in range(4):
                nc.sync.dma_start(out=oth_sb[:, 0, ks, :], in_=oth_d[0, ks])
            for ks in range(4):
                nc.sync.dma_start(out=wvo_sb[:, ks, :], in_=wvoT_d[ks])
            for i in range(1, 3):
                for ks in range(4):
                    nc.sync.dma_start(out=wk_sb[:, i, ks, :], in_=wkT_d[i, ks])
                    nc.sync.dma_start(out=wq_sb[:, i, ks, :], in_=wqT_d[i, ks])
            for ks in range(4):
                nc.sync.dma_start(out=oth_sb[:, 1, ks, :], in_=oth_d[1, ks])

            # Small consts: DMA to staging, then re-own on the consuming
            # engine (DVE / ACT) so consumers need no cross-engine const wait.
            dvec_st = const.tile([128, 6], F32)
            nc.gpsimd.dma_start(out=dvec_st, in_=dvec_d[:])
            wobv_st = const.tile([128, 384], F32)
            nc.gpsimd.dma_start(out=wobv_st, in_=wobv_d[:].partition_broadcast(128))
            avec_st = const.tile([128, 4], F32)
            nc.gpsimd.dma_start(out=avec_st, in_=avec_d[:])
            dvec = const.tile([128, 6], F32)
            nc.vector.tensor_copy(dvec, dvec_st)
            wobv_sb = const.tile([128, 384], F32)
            nc.vector.tensor_copy(wobv_sb, wobv_st)
            avec = const.tile([128, 4], F32)
            nc.scalar.activation(out=avec, in_=avec_st, func=AF.Copy)
            bqv_sb = dvec[:, 0:3]
            bkv_sb = dvec[:, 3:6]

            # xt (attention output) slabs + own h1 slab, zero-padded 34x34
            xt_sl = []
            for i in range(3):
                t = xtp.tile([128, HP, HP], BF16, name=f"xt{i}")
                nc.vector.memset(t, 0.0)
                xt_sl.append(t)
            h1_own = xtp.tile([128, HP, HP], BF16, name="h1own")
            nc.vector.memset(h1_own, 0.0)
            h1_sl = [xtp.tile([128, HP, HP], BF16, name=f"h1sl{k}") for k in range(4)]
            # conv1 accumulator (own 128 out channels, fp32)
            acc1 = xtp.tile([128, S], F32, name="acc1")

            # Semaphore warmers: absorb const-DMA + memset waits into each
            # engine's observed clock so later compute ops need <=1 wait.
            warm = const.tile([128, 1], F32)
            nc.vector.tensor_copy(warm, dvec[:, 0:1])
            warm2 = const.tile([128, 1], F32)
            nc.scalar.activation(out=warm2, in_=warm, func=AF.Copy)

            # k/q per branch (with biases added), uT tiles ([u | 1] cols)
            k_sb = kq.tile([128, 3, S], BF16)
            q_sb = kq.tile([128, 3, S], BF16)
            uT = [kq.tile([128, 3, 4, 33], BF16, name=f"uT{t}") for t in range(8)]

            qsrc = [oth_sb[:, 0], x_sb, oth_sb[:, 1]]

            def proj_kq(i):
                k_ps = scps.tile([128, S], F32, name="kq_ps", tag="sc")
                for s in range(2):
                    for ks in range(4):
                        nc.tensor.matmul(
                            k_ps[:, 512 * s : 512 * (s + 1)],
                            lhsT=wk_sb[:, i, ks, :],
                            rhs=x_sb[:, ks, 512 * s : 512 * (s + 1)],
                            start=(ks == 0),
                            stop=(ks == 3),
                        )
                nc.vector.tensor_scalar_add(k_sb[:, i, :], k_ps, bkv_sb[:, i : i + 1])

                q_ps = scps.tile([128, S], F32, name="kq_ps2", tag="sc")
                for s in range(2):
                    for ks in range(4):
                        nc.tensor.matmul(
                            q_ps[:, 512 * s : 512 * (s + 1)],
                            lhsT=wq_sb[:, i, ks, :],
                            rhs=qsrc[i][:, ks, 512 * s : 512 * (s + 1)],
                            start=(ks == 0),
                            stop=(ks == 3),
                        )
                nc.vector.tensor_scalar_add(q_sb[:, i, :], q_ps, bqv_sb[:, i : i + 1])

            def proj_u(t):
                u_ps = yps.tile([128, 384], F32, name="u_ps", tag="y")
                for ks in range(4):
                    nc.tensor.matmul(
                        u_ps,
                        lhsT=x_sb[:, ks, 128 * t : 128 * (t + 1)],
                        rhs=wvo_sb[:, ks, :],
                        start=(ks == 0),
                        stop=(ks == 3),
                    )
                nc.vector.memset(uT[t][:, :, :, 32:33], 1.0)
                nc.vector.tensor_add(
                    uT[t][:, :, :, 0:32],
                    u_ps.rearrange("p (i h d) -> p i h d", i=3, h=4),
                    wobv_sb.rearrange("p (i h d) -> p i h d", i=3, h=4),
                )

            # ---- conv weights (after initial DMAs) -----------------------
            convw = ctx.enter_context(tc.tile_pool(name="convw", bufs=1))
            c1w_sb = [
                [convw.tile([128, 9, 128], BF16, name=f"c1w{i}_{m}") for m in range(4)]
                for i in range(3)
            ]
            c2w_sb = [convw.tile([128, 9, 128], BF16, name=f"c2w{m}") for m in range(4)]
            for m in range(4):
                nc.sync.dma_start(out=c1w_sb[0][m], in_=c1wT_d[0, m])
            for m in range(4):
                nc.sync.dma_start(out=c2w_sb[m], in_=c2wT_d[m])
            for i in range(1, 3):
                for m in range(4):
                    nc.sync.dma_start(out=c1w_sb[i][m], in_=c1wT_d[i, m])

            # DRAM staging for the ReduceScatters (bf16 partials)
            part1_d = [dram.tile([512, S], BF16, name=f"p1_{i}") for i in range(3)]
            rs1_d = [dram.tile([128, S], BF16, name=f"r1_{i}") for i in range(3)]
            H1CH = [(0, 18), (18, 16)]  # h1 AllGather chunks in padded rows
            h1st_d = [dram.tile([128, H1CH[a][1] * HP], BF16, name=f"h1s{a}") for a in range(2)]
            h1g_d = [dram.tile([512, H1CH[a][1] * HP], BF16, name=f"h1g{a}") for a in range(2)]
            # branch-2 partials split into two contiguous DRAM tiles so the
            # two RS chunks have contiguous inputs (cols [0:704) and [704:))
            p2c_d = [dram.tile([512, 704], BF16, name="p2a"), dram.tile([512, 320], BF16, name="p2b")]
            r2c_d = [dram.tile([128, 704], BF16, name="r2a"), dram.tile([128, 320], BF16, name="r2b")]

            # ------------- conv partial building blocks -------------------
            def conv1_unit(i, m, n):
                """Local conv1 partial: branch i's own 128 in-ch, out m-tile,
                spatial half n; result staged to DRAM (bf16) for the RS."""
                ps = cvps.tile([128, 512], F32, name="cv", tag="cv")
                for dy in range(3):
                    for dx in range(3):
                        nc.tensor.matmul(
                            ps,
                            lhsT=c1w_sb[i][m][:, dy * 3 + dx, :],
                            rhs=xt_sl[i][:, 16 * n + dy : 16 * n + dy + 16, dx : dx + 32],
                            start=(dy == 0 and dx == 0),
                            stop=(dy == 2 and dx == 2),
                        )
                st = stg.tile([128, 512], BF16, name="c1st")
                nc.vector.tensor_copy(st, ps)
                nc.sync.dma_start(
                    out=part1_d[i][128 * m : 128 * (m + 1), 512 * n : 512 * (n + 1)],
                    in_=st,
                )

            def rs1(i):
                nc.gpsimd.collective_compute(
                    "ReduceScatter",
                    mybir.AluOpType.add,
                    replica_groups=GROUPS,
                    ins=[part1_d[i][:]],
                    outs=[rs1_d[i][:]],
                )
                rsb = stg.tile([128, S], BF16, name="rsb")
                nc.sync.dma_start(out=rsb, in_=rs1_d[i][:])
                if i == 0:
                    nc.vector.tensor_copy(acc1, rsb)
                else:
                    nc.vector.tensor_add(acc1, rsb, acc1)

            def attention_pair(i, pr, fillers, fst, fire):
                """Head pair pr of branch i as one software-pipelined t-loop:
                scores(t)+exp(t) emitted one step ahead of y(t); filler units
                (conv1 partial blocks / projections) keep PE busy while ACT
                grinds through the exps."""
                pts = {}

                def emit_sc(t):
                    for hh in range(2):
                        h = 2 * pr + hh
                        sc = scps.tile([128, S], F32, name="sc", tag="sc")
                        p0 = 32 * h
                        for s in range(2):
                            nc.tensor.matmul(
                                sc[:, 512 * s : 512 * (s + 1)],
                                lhsT=k_sb[p0 : p0 + 32, i, 128 * t : 128 * (t + 1)],
                                rhs=q_sb[p0 : p0 + 32, i, 512 * s : 512 * (s + 1)],
                                start=True,
                                stop=True,
                                tile_position=(p0, 0),
                            )
                        ptt = pt.tile([128, S], BF16, name="ptt")
                        nc.scalar.activation(
                            out=ptt, in_=sc, func=AF.Exp, scale=float(ISQD)
                        )
                        pts[(hh, t)] = ptt

                def emit_y(ytl, s, t):
                    for hh in range(2):
                        h = 2 * pr + hh
                        nc.tensor.matmul(
                            ytl[hh],
                            lhsT=uT[t][:, i, h, :],
                            rhs=pts[(hh, t)][:, 512 * s : 512 * (s + 1)],
                            start=(t == 0),
                            stop=(t == 7),
                        )

                def normalize(ytl, s):
                    # recip_approx_fast requires SBUF partition-0 input and
                    # partition_broadcast a partition-0 source (both HW-
                    # verified), so copy each denominator row down first.
                    for hh in range(2):
                        h = 2 * pr + hh
                        dh = rcp.tile([1, 512], F32, name="dh")
                        nc.vector.tensor_copy(dh, ytl[hh][32:33, :])
                        rch = rcp.tile([1, 512], F32, name="rch")
                        nc.vector.reciprocal_approx_fast(out=rch, in_=dh)
                        rcb = rcp.tile([32, 512], F32, name="rcb")
                        nc.gpsimd.partition_broadcast(rcb, rch)
                        p0 = 32 * h
                        nc.vector.tensor_mul(
                            xt_sl[i][p0 : p0 + 32, 1 + 16 * s : 17 + 16 * s, 1:33],
                            ytl[hh][0:32, :].rearrange("p (a b) -> p a b", b=32),
                            rcb.rearrange("p (a b) -> p a b", b=32),
                        )

                # s=0 accumulates inside the t-loop (PE trails ACT by one
                # exp); s=1 re-reads the kept pts tiles in a second pass.
                # Emission order per t puts never-stalling filler work and
                # the y matmuls BEFORE the sc matmuls: the PE queue is in-
                # order, so a sc stalled on an ACT exp must not sit in front
                # of already-runnable work (head-of-line blocking would also
                # keep dropping the PE out of its full p-state).
                y0 = [yps.tile([33, 512], F32, name=f"y0_{hh}", tag="y") for hh in range(2)]
                for t in range(8):
                    if t in fire and fst[0] < len(fillers):
                        fillers[fst[0]]()
                        fst[0] += 1
                    if t >= 1:
                        emit_y(y0, 0, t - 1)
                    emit_sc(t)
                emit_y(y0, 0, 7)
                normalize(y0, 0)
                y1 = [yps.tile([33, 512], F32, name=f"y1_{hh}", tag="y") for hh in range(2)]
                for t in range(8):
                    emit_y(y1, 1, t)
                normalize(y1, 1)

            # ---------------- emission schedule ---------------------------
            # proj for branch 0 + uT first so attention can start early
            proj_kq(0)
            for t in range(8):
                proj_u(t)

            for i in range(3):
                # filler units during branch i's attention: branch 0 gets the
                # remaining projections, branches 1/2 get the LOCAL conv1
                # partial blocks of branch i-1 (no collective dependency).
                if i == 0:
                    fillers = [lambda: proj_kq(1), lambda: proj_kq(2)]
                    fire0, fire1 = {2, 5}, {2, 5}
                else:
                    fillers = [
                        (lambda m=m, n=n: conv1_unit(i - 1, m, n))
                        for n in range(2)
                        for m in range(4)
                    ] + [lambda: rs1(i - 1)]
                    fire0, fire1 = {1, 2, 3, 4, 5}, {1, 2, 3, 4, 5}
                fst = [0]
                attention_pair(i, 0, fillers, fst, fire0)
                attention_pair(i, 1, fillers, fst, fire1)
                while fst[0] < len(fillers):
                    fillers[fst[0]]()
                    fst[0] += 1

            # ---- tail ----------------------------------------------------
            # conv1 partial for branch 2 in 3 row-blocks so the RS can go in
            # 2 column chunks; h1 relu + 2-chunk h1 AllGather; conv2 = full
            # 512-channel contraction for the own 128 outputs, per n-half.
            C1R3 = [(0, 11), (11, 11), (22, 10)]

            def conv1_unit3(m, blk):
                r0, nr = C1R3[blk]
                ps = cvps.tile([128, 512], F32, name="cv3", tag="cv")
                for dy in range(3):
                    for dx in range(3):
                        nc.tensor.matmul(
                            ps[:, : nr * 32],
                            lhsT=c1w_sb[2][m][:, dy * 3 + dx, :],
                            rhs=xt_sl[2][:, r0 + dy : r0 + dy + nr, dx : dx + 32],
                            start=(dy == 0 and dx == 0),
                            stop=(dy == 2 and dx == 2),
                        )
                st = stg.tile([128, 512], BF16, name="c1st3")
                nc.vector.tensor_copy(st[:, : nr * 32], ps[:, : nr * 32])
                if blk < 2:
                    dst = p2c_d[0][128 * m : 128 * (m + 1), 32 * r0 : 32 * (r0 + nr)]
                else:
                    dst = p2c_d[1][128 * m : 128 * (m + 1), 0 : 32 * nr]
                nc.sync.dma_start(out=dst, in_=st[:, : nr * 32])

            # rs1(2) in two chunks: cols [0:704) and [704:1024)
            RCH = [(0, 704), (704, 320)]

            def rs1c(a):
                nc.gpsimd.collective_compute(
                    "ReduceScatter",
                    mybir.AluOpType.add,
                    replica_groups=GROUPS,
                    ins=[p2c_d[a][:]],
                    outs=[r2c_d[a][:]],
                )

            def rs1c_unload(a):
                c0, nn = RCH[a]
                rsb = stg.tile([128, 704], BF16, name="rsb3")
                nc.sync.dma_start(out=rsb[:, :nn], in_=r2c_d[a][:])
                nc.vector.tensor_add(
                    acc1[:, c0 : c0 + nn], rsb[:, :nn], acc1[:, c0 : c0 + nn]
                )
                # relu rows covered by this chunk (valid rows a==0: 0..16,
                # a==1: 17..31)
                v0, v1 = (0, 17) if a == 0 else (17, 32)
                nc.scalar.activation(
                    out=h1_own[:, 1 + v0 : 1 + v1, 1:33],
                    in_=acc1[:, 32 * v0 : 32 * v1].rearrange("p (a b) -> p a b", b=32),
                    func=AF.Relu,
                    bias=avec[:, 1:2],
                    scale=avec[:, 0:1],
                )
                # stage + AllGather padded rows chunk
                pr0, nr = H1CH[a]
                nc.sync.dma_start(
                    out=h1st_d[a][:],
                    in_=h1_own[:, pr0 : pr0 + nr, :].rearrange("p a b -> p (a b)"),
                )
                nc.gpsimd.collective_compute(
                    "AllGather",
                    mybir.AluOpType.bypass,
                    replica_groups=GROUPS,
                    ins=[h1st_d[a][:]],
                    outs=[h1g_d[a][:]],
                )
                for k in range(4):
                    nc.sync.dma_start(
                        out=h1_sl[k][:, pr0 : pr0 + nr, :].rearrange("p a b -> p (a b)"),
                        in_=h1g_d[a][128 * k : 128 * (k + 1), :],
                    )

            for blk in range(2):
                for m in range(4):
                    conv1_unit3(m, blk)
            rs1c(0)
            for m in range(4):
                conv1_unit3(m, 2)
            rs1c(1)
            rs1c_unload(0)
            rs1c_unload(1)

            oout = stg.tile([128, S], F32, name="oout", bufs=1)
            for n in range(2):
                ps2 = cvps.tile([128, 512], F32, name=f"cv2_{n}", tag="cv")
                for k in range(4):
                    for dy in range(3):
                        for dx in range(3):
                            nc.tensor.matmul(
                                ps2,
                                lhsT=c2w_sb[k][:, dy * 3 + dx, :],
                                rhs=h1_sl[k][
                                    :, 16 * n + dy : 16 * n + dy + 16, dx : dx + 32
                                ],
                                start=(k == 0 and dy == 0 and dx == 0),
                                stop=(k == 3 and dy == 2 and dx == 2),
                            )
                nc.scalar.activation(
                    out=oout[:, 512 * n : 512 * (n + 1)],
                    in_=ps2,
                    func=AF.Relu,
                    bias=avec[:, 3:4],
                    scale=avec[:, 2:3],
                )
                nc.sync.dma_start(
                    out=out_d[:, 512 * n : 512 * (n + 1)],
                    in_=oout[:, 512 * n : 512 * (n + 1)],
                )

    nc.finalize()
    return nc


def _f(x):
    return np.ascontiguousarray(x, dtype=np.float32)


def _bf(x):
    return np.ascontiguousarray(np.asarray(x, dtype=np.float32).astype(ml_dtypes.bfloat16))


def prepare_core_inputs(inp):
    """Build the 8 per-core input dicts from the full-problem inputs."""
    inp = {k: np.asarray(v, dtype=np.float64) for k, v in inp.items()}
    x = inp["x"].reshape(B, C, S)
    xp = inp["x_prev"].reshape(B, C, S)
    xn = inp["x_next"].reshape(B, C, S)

    bn1s_full = inp["bn1g"] / np.sqrt(inp["bn1v"] + EPS)
    bn1b_full = inp["bn1b"] - inp["bn1m"] * bn1s_full
    bn2s_full = inp["bn2g"] / np.sqrt(inp["bn2v"] + EPS)
    bn2b_full = inp["bn2b"] - inp["bn2m"] * bn2s_full

    per_g = []
    for g in range(4):
        sl = slice(128 * g, 128 * (g + 1))
        wqT = np.stack(
            [
                np.stack([inp["Wq"][i][sl, 128 * k : 128 * (k + 1)].T for k in range(4)])
                for i in range(3)
            ]
        )
        wkT = np.stack(
            [
                np.stack([inp["Wk"][i][sl, 128 * k : 128 * (k + 1)].T for k in range(4)])
                for i in range(3)
            ]
        )
        bqv = np.stack([inp["bq"][i][sl] for i in range(3)], axis=1)
        bkv = np.stack([inp["bk"][i][sl] for i in range(3)], axis=1)

        att_s = np.stack(
            [inp["bng"][i][sl] / np.sqrt(inp["bnv"][i][sl] + EPS) for i in range(3)]
        )  # (3,128)
        xtb = np.stack(
            [
                inp["bnb"][i][sl] + (inp["bo"][i][sl] - inp["bnm"][i][sl]) * att_s[i]
                for i in range(3)
            ]
        )  # (3,128)  full BN bias for this slab, folded into wobv below

        wvo_rows = []
        wobv_row = []
        for i in range(3):
            for hl in range(4):
                hg = 4 * g + hl
                wv_h = inp["Wv"][i][32 * hg : 32 * (hg + 1), :]  # (32, 512)
                bv_h = inp["bv"][i][32 * hg : 32 * (hg + 1)]
                wo_h = inp["Wo"][i, hg]  # (32, 32)
                sc = att_s[i][32 * hl : 32 * (hl + 1)]  # (32,)
                wvo_rows.append(sc[:, None] * (wo_h @ wv_h))
                # fold the full BN/bias offset in: y/den then equals bn'd out
                wobv_row.append(sc * (wo_h @ bv_h) + xtb[i][32 * hl : 32 * (hl + 1)])
        wvo_all = np.concatenate(wvo_rows, axis=0)  # (384, 512)
        wobv = np.concatenate(wobv_row)[None, :]  # (1, 384)
        wvoT = np.stack([wvo_all[:, 128 * k : 128 * (k + 1)].T for k in range(4)])

        # conv1 weights: own 128 in-ch (per branch) -> all 512 out (4 m-tiles)
        c1wT = np.stack(
            [
                np.stack(
                    [
                        inp["c1w"][
                            128 * m : 128 * (m + 1),
                            512 * i + 128 * g : 512 * i + 128 * (g + 1),
                        ]
                        .transpose(1, 2, 3, 0)
                        .reshape(128, 9, 128)
                        for m in range(4)
                    ]
                )
                for i in range(3)
            ]
        )
        # conv2 weights: full 512-ch contraction -> own 128 out
        c2wT = np.stack(
            [
                inp["c2w"][sl, 128 * k : 128 * (k + 1)]
                .transpose(1, 2, 3, 0)
                .reshape(128, 9, 128)
                for k in range(4)
            ]
        )
        avec = np.stack(
            [bn1s_full[sl], bn1b_full[sl], bn2s_full[sl], bn2b_full[sl]], axis=1
        )  # (128, 4)

        per_g.append(
            dict(
                wqT=_bf(wqT), wkT=_bf(wkT), wvoT=_bf(wvoT),
                wobv=_f(wobv), c1wT=_bf(c1wT), c2wT=_bf(c2wT),
                dvec=_f(np.concatenate([bqv, bkv], axis=1)),
                avec=_f(avec),
            )
        )

    in_maps = []
    for c in range(NCORES):
        b, g = c // 4, c % 4
        d = dict(per_g[g])
        d["x4"] = _bf(x[b].reshape(4, 128, S))
        d["oth"] = _bf(np.stack([xn[b].reshape(4, 128, S), xp[b].reshape(4, 128, S)]))
        in_maps.append(d)
    return in_maps


_NC_CACHE = {}


def get_nc():
    if "nc" not in _NC_CACHE:
        _NC_CACHE["nc"] = build_nc()
    return _NC_CACHE["nc"]


def assemble(results):
    out = np.zeros((B, C, H, W), dtype=np.float32)
    for c in range(NCORES):
        b, g = c // 4, c % 4
        out[b, 128 * g : 128 * (g + 1)] = results[c]["out"].reshape(128, H, W)
    return out


def kernel(**inputs):
    nc = get_nc()
    in_maps = prepare_core_inputs(inputs)
    res = run_bass_kernel_spmd(nc, in_maps, list(range(NCORES)))
    return assemble(res.results)
